# revision 1
# baseline (speedup 1.0000x reference)
"""Trainium2 Bass kernel for nn_Cross_AgentAttention.

Data-parallel over batch B=8 across 8 NeuronCores; params replicated.

Per-core algorithm (feature-major (c, n) layout, exploiting that
q = guidmap @ q_w + q_b is rank-1):
  - v = x @ Wv via fp32r matmuls (TF32-grade, full PE rate), evacuated as
    fp8 hi+lo planes (hi = fp8(v), lo = fp8(v-hi)) with zero-padded y
    borders.
  - agent->kv attention collapses to kw[h,i] = w_h . k_h[i], computed as
    x @ Mkw (folded on host); logits = scale*gbar_a*kw[h,i] + PB, where
    row-constant terms cancel in softmax.  attn = exp(logits) stored fp8.
  - query->agent attention collapses to a rank-1 logit map
    lq[i,(h,a)] = g_i * u[(h,a)] (+r) + ABt; qnt stored fp8.
  - agent_v via fp8 DoubleRow matmuls over transposed attn/v chunks
    (both q-chunks of a pair packed in the two K-tiles of one DR op).
  - depthwise 3x3 conv: every tap is fp8 DoubleRow matmuls against the
    padded v hi/lo planes; weight fp8 residuals get their own paired DR
    ops, so the conv is bf16-grade accurate at fp8 double-pump rate.
  - x-wraparound border corrections batched once per branch.
  - attn-output matmul = one DR op per strip (avsel hi/lo planes).
"""
import numpy as np
import ml_dtypes

import concourse.bass as bass
import concourse.bacc as bacc
import concourse.mybir as mybir
from concourse.tile import TileContext
from concourse.ap import AP
from concourse.bass_utils import run_bass_kernel_spmd

F32 = mybir.dt.float32
F32R = mybir.dt.float32r
BF16 = mybir.dt.bfloat16
FP8 = mybir.dt.float8e4
BF = ml_dtypes.bfloat16
F8 = ml_dtypes.float8_e4m3
DRM = mybir.MatmulPerfMode.DoubleRow

DIM = 256
HEADS = 8
AGENT = 16
H = W = 64
B = 8
N = H * W                 # 4096
HD = DIM // HEADS         # 32
SCALE = HD ** -0.5
PS = 4
NT = 8                    # n-tiles of 512
NTW = N // NT             # 512
NCH = 32                  # n-chunks of 128
ROWS_PER_NT = NTW // W    # 8 image rows per n-tile
VPAD = 128                # zero pad on each side of each v plane
VPLANE = VPAD + N + VPAD  # 4352
# conv tap shifts, row-major (dy,dx) in (-1,0,1)^2
TAPS = [dy * W + dx for dy in (-1, 0, 1) for dx in (-1, 0, 1)]

AL = mybir.AluOpType
AF = mybir.ActivationFunctionType


# ----------------------------------------------------------------------------
# host precompute
# ----------------------------------------------------------------------------

def _bilinear_matrix(n_in, n_out):
    U = np.zeros((n_out, n_in), dtype=np.float64)
    s = n_in / n_out
    for o in range(n_out):
        x = (o + 0.5) * s - 0.5
        x0 = int(np.floor(x))
        t = x - x0
        for i, wt in ((x0, 1.0 - t), (x0 + 1, t)):
            ic = min(max(i, 0), n_in - 1)
            U[o, ic] += wt
    return U.astype(np.float32)


def _host_precompute(kv_w, kv_b, q_w, q_b, proj_w, proj_b, dwc_w, dwc_b,
                     an_bias, na_bias, ah_bias, aw_bias, ha_bias, wa_bias):
    c = DIM
    w = q_w[0]
    beta = q_b
    U = _bilinear_matrix(PS, H)

    # logits-matmul constant operands: logits = LOG^T @ RHS with
    # LOG rows = [Eg-gbar(8, device) | EgC(8) | an_tbl(16) | ahT(64) | awT(64)]
    # RHS rows = [kw(8, device) | kbeta(8, device) | UU(16) | Yind(64) | Xind(64)]
    an_tbl = an_bias.reshape(HEADS * AGENT, PS * PS).T.astype(np.float32)      # (16, 128)
    UU = np.einsum("yr,xc->rcyx", U, U).reshape(PS * PS, N).astype(np.float32)  # (16, 4096)
    ahT = ah_bias[0][..., 0].reshape(HEADS * AGENT, H).T.astype(np.float32)    # (64, 128)
    awT = aw_bias[0][:, :, 0, :].reshape(HEADS * AGENT, W).T.astype(np.float32)
    Yind = np.kron(np.eye(H, dtype=np.float32), np.ones((1, W), np.float32))    # (64, 4096)
    Xind = np.concatenate([np.eye(W, dtype=np.float32)] * H, axis=1)            # (64, 4096)

    na_up = np.einsum("yr,harc,xc->hayx", U, na_bias.reshape(HEADS, AGENT, PS, PS), U)
    ab = na_up.reshape(HEADS, AGENT, N).transpose(0, 2, 1)
    ab = ab + (ha_bias[0] + wa_bias[0]).reshape(HEADS, N, AGENT)
    ABt = ab.transpose(1, 0, 2).reshape(N, HEADS * AGENT).astype(np.float32)

    wk = kv_w[:, :c]
    Mkw = np.stack([(wk[:, h*HD:(h+1)*HD] * w[None, h*HD:(h+1)*HD]).sum(1)
                    for h in range(HEADS)], axis=1)
    Mkb = np.stack([(wk[:, h*HD:(h+1)*HD] * beta[None, h*HD:(h+1)*HD]).sum(1)
                    for h in range(HEADS)], axis=1)
    MM = np.concatenate([Mkw, Mkb], axis=1).astype(np.float32)      # (256, 16)

    hw2 = np.array([(w[h*HD:(h+1)*HD]**2).sum() for h in range(HEADS)], np.float32)
    wb = np.array([(w[h*HD:(h+1)*HD]*beta[h*HD:(h+1)*HD]).sum() for h in range(HEADS)], np.float32)
    bb = np.array([(beta[h*HD:(h+1)*HD]**2).sum() for h in range(HEADS)], np.float32)
    # qrows: [s*hw2 | s*wb | s*bb] repeated per agent -> (1, 384)
    qrows = np.concatenate([np.repeat(SCALE * hw2, AGENT),
                            np.repeat(SCALE * wb, AGENT),
                            np.repeat(SCALE * bb, AGENT)])[None, :].astype(np.float32)

    Wv = kv_w[:, c:].astype(np.float32)                              # (256, 256)
    bv = kv_b[c:].astype(np.float32)

    headmask = np.zeros((HEADS * AGENT, c), np.float32)
    for h in range(HEADS):
        headmask[h*AGENT:(h+1)*AGENT, h*HD:(h+1)*HD] = 1.0

    # EgC: constant rows 8..15 of Eg (selector for the k-beta stream)
    EgC = np.zeros((8, 128), np.float32)
    for h in range(HEADS):
        EgC[h, h*AGENT:(h+1)*AGENT] = SCALE
    HB8 = EgC.copy()   # same pattern masks the gbar broadcast into Eg rows 0..7

    # DIAGW: fp8 hi/lo diagonal tap matrices.
    # layout [k=128, plane(2: hi,lo), tap(10: 0..8 + zero), pt(2), m=128]
    dwc9 = dwc_w.reshape(c, 9).astype(np.float32)
    w_hi = dwc9.astype(F8).astype(np.float32)
    w_lo = (dwc9 - w_hi).astype(F8).astype(np.float32)
    DIAGW = np.zeros((128, 2, 10, 2, 128), np.float32)
    for plane, wsrc in ((0, w_hi), (1, w_lo)):
        for t in range(9):
            for pt in range(2):
                np.fill_diagonal(DIAGW[:, plane, t, pt, :], wsrc[pt*128:(pt+1)*128, t])

    BLK = np.zeros((16, 128), np.float32)                            # gbar -> (h,a) expand
    for a in range(16):
        BLK[a, a::16] = 1.0

    NEG9 = np.zeros((128, 18), np.float32)
    for pt in range(2):
        NEG9[:, pt*9:(pt+1)*9] = -dwc9[pt*128:(pt+1)*128, :]

    projb = np.stack([proj_b[:128], proj_b[128:]], axis=1).astype(np.float32)  # (128, 2)
    bvcol = np.stack([bv[:128], bv[128:]], axis=1).astype(np.float32)          # (128, 2)
    dwbcol = np.stack([dwc_b[:128], dwc_b[128:]], axis=1).astype(np.float32)   # (128, 2)

    flags = dict(
        has_qb=bool(np.any(q_b != 0)),
        has_kvb_v=bool(np.any(bv != 0)),
        has_dwcb=bool(np.any(dwc_b != 0)),
        has_projb=bool(np.any(proj_b != 0)),
    )
    qmeta = dict(qrows=np.concatenate([np.repeat(SCALE * hw2, AGENT),
                                       np.repeat(SCALE * wb, AGENT),
                                       np.repeat(SCALE * bb, AGENT)]).reshape(3, 128),
                 EgC=EgC)

    LOGC_A = np.zeros((128, 128), np.float32)
    LOGC_A[8:16] = EgC
    LOGC_A[16:32] = an_tbl
    LOGC_A[32:96] = ahT
    LOGC_A[96:128] = awT[0:32]
    LOGC_B = np.zeros((128, 128), np.float32)        # plane-1 lhsT, zero-padded
    LOGC_B[0:32] = awT[32:64]
    RHSC_A = np.concatenate([UU, Yind, Xind[0:32]], axis=0)   # (112, 4096)
    RHSC_B = Xind[32:64].copy()                               # (32, 4096)

    # SMALL_BF (128, 144): [ident 0:128 | i16 128:144]
    SMALL_BF = np.zeros((128, 144), np.float32)
    SMALL_BF[:, 0:128] = np.eye(128, dtype=np.float32)
    SMALL_BF[0:16, 128:144] = np.eye(16, dtype=np.float32)
    # SMALL_F32 (128, 24): [neg9 0:18 | projb 18:20 | bvcol 20:22 | dwb 22:24]
    SMALL_F32 = np.concatenate([NEG9, projb, bvcol, dwbcol], axis=1)

    IDENT8 = np.eye(128, dtype=np.float32)
    IDF32 = np.eye(128, dtype=np.float32)

    params = dict(
        LOGC_A=LOGC_A.astype(F8), LOGC_B=LOGC_B.astype(F8),
        RHSC_A=RHSC_A.astype(F8), RHSC_B=RHSC_B.astype(F8),
        ABt=ABt.astype(F8),
        WVMM=np.concatenate([Wv[0:128], Wv[128:256], MM[0:128], MM[128:256]],
                            axis=1).astype(BF),
        PW=proj_w.astype(np.float32).astype(BF),
        DIAGW=DIAGW.astype(F8), HM=headmask.astype(BF),
        IDENT8=IDENT8.astype(F8), IDF32=IDF32.astype(BF),
        SMALL_BF=SMALL_BF.astype(BF), SMALL_F32=SMALL_F32.astype(np.float32),
    )
    params["qmeta"] = qmeta
    return params, flags


# ----------------------------------------------------------------------------
# device kernel builder
# ----------------------------------------------------------------------------

def _build(flags):
    nc = bacc.Bacc(None, target_bir_lowering=False, debug=False)

    # ---- DRAM I/O ----
    x_in = [nc.dram_tensor(f"x{m+1}", [DIM, N], BF16, kind="ExternalInput") for m in range(2)]
    gcols = nc.dram_tensor("gcols", [128, NCH], F32, kind="ExternalInput")
    dEG8 = nc.dram_tensor("EG8", [8, 128], FP8, kind="ExternalInput")
    dU128 = nc.dram_tensor("U128", [128, 128], BF16, kind="ExternalInput")
    dR128 = nc.dram_tensor("R128", [128, 128], BF16, kind="ExternalInput")
    dLOGA = nc.dram_tensor("LOGC_A", [128, 128], FP8, kind="ExternalInput")
    dLOGB = nc.dram_tensor("LOGC_B", [128, 128], FP8, kind="ExternalInput")
    dRHSA = nc.dram_tensor("RHSC_A", [112, N], FP8, kind="ExternalInput")
    dRHSB = nc.dram_tensor("RHSC_B", [32, N], FP8, kind="ExternalInput")
    dABt = nc.dram_tensor("ABt", [N, 128], FP8, kind="ExternalInput")
    dWVMM = nc.dram_tensor("WVMM", [128, 544], BF16, kind="ExternalInput")
    dPW = nc.dram_tensor("PW", [DIM, DIM], BF16, kind="ExternalInput")
    dDIAGW = nc.dram_tensor("DIAGW", [128, 2 * 10 * 2 * 128], FP8, kind="ExternalInput")
    dHM = nc.dram_tensor("HM", [128, DIM], BF16, kind="ExternalInput")
    dI8 = nc.dram_tensor("IDENT8", [128, 128], FP8, kind="ExternalInput")
    dIDF = nc.dram_tensor("IDF32", [128, 128], BF16, kind="ExternalInput")
    dSBF = nc.dram_tensor("SMALL_BF", [128, 144], BF16, kind="ExternalInput")
    dSF32 = nc.dram_tensor("SMALL_F32", [128, 24], F32, kind="ExternalInput")
    o_out = [nc.dram_tensor(f"o{m+1}", [DIM, N], BF16, kind="ExternalOutput") for m in range(2)]

    with TileContext(nc) as tc:
        with (
            tc.tile_pool(name="wpool", bufs=1) as wp,          # weights/consts
            tc.tile_pool(name="big", bufs=1) as bigp,          # big per-branch tensors
            tc.tile_pool(name="xpool", bufs=2) as xp,          # input prefetch
            tc.tile_pool(name="small", bufs=3) as sp,          # rotating small tiles
            tc.tile_pool(name="ps_big", bufs=3, space="PSUM") as psb,    # (128,512)
            tc.tile_pool(name="ps_half", bufs=3, space="PSUM") as psh,   # (128,256)
            tc.tile_pool(name="ps_sm", bufs=1, space="PSUM") as pssm,    # (128,128)
            tc.tile_pool(name="ps_av", bufs=1, space="PSUM") as psav,
        ):
            # ---------------- critical-path DMAs first ----------------
            # tiny q-path seeds first (they head the DVE queue), then the
            # v-matmul operands strip-chunked so the first matmul starts early
            wvmm = wp.tile([128, 544], BF16)
            nc.sync.dma_start(wvmm[:], dWVMM[:])
            wv_f = wvmm[:, 0:512]

            xts = []
            smallbf = wp.tile([128, 144], BF16)
            gcols_t = wp.tile([128, NCH], F32)
            smallf = wp.tile([128, 24], F32)
            for m in range(2):
                xtm = [xp.tile([128, N], BF16, tag=f"x{m}{pt}", name=f"xt{m}{pt}", bufs=1) for pt in range(2)]
                if m == 0:
                    for t2 in range(NT // 2):
                        for pt in range(2):
                            nc.sync.dma_start(xtm[pt][:, t2*2*NTW:(t2+1)*2*NTW],
                                              x_in[0][pt*128:(pt+1)*128, t2*2*NTW:(t2+1)*2*NTW])
                    nc.sync.dma_start(smallbf[:], dSBF[:])
                    nc.sync.dma_start(gcols_t[:], gcols[:])
                    nc.sync.dma_start(smallf[:], dSF32[:])
                xts.append(xtm)

            mm_f = wvmm[:, 512:544]

            u128 = wp.tile([128, 128], BF16)
            nc.sync.dma_start(u128[:], dU128[:])
            loga = wp.tile([128, 128], FP8)
            nc.sync.dma_start(loga[8:128, :], dLOGA[8:128, :])
            nc.sync.dma_start(loga[0:8, :], dEG8[:])
            logb = wp.tile([32, 128], FP8)
            nc.sync.dma_start(logb[:], dLOGB[0:32, :])
            if flags["has_qb"]:
                r128 = wp.tile([128, 128], BF16)
                nc.sync.dma_start(r128[:], dR128[:])

            abt = bigp.tile([128, NCH * 128], FP8, tag="attn", bufs=2)
            nc.sync.dma_start(
                abt[:].rearrange("p (j f) -> p j f", j=NCH),
                dABt[:].rearrange("(j p) f -> p j f", j=NCH))

            for t2 in range(NT // 2):
                for pt in range(2):
                    nc.sync.dma_start(xts[1][pt][:, t2*2*NTW:(t2+1)*2*NTW],
                                      x_in[1][pt*128:(pt+1)*128, t2*2*NTW:(t2+1)*2*NTW])
            ident = smallbf[:, 0:128]
            neg9 = smallf[:, 0:18]
            projb = smallf[:, 18:20]
            bvcol = smallf[:, 20:22]
            dwbcol = smallf[:, 22:24]

            ident8 = wp.tile([128, 128], FP8)
            nc.sync.dma_start(ident8[:], dI8[:])
            idf = wp.tile([128, 128], BF16)
            nc.sync.dma_start(idf[:], dIDF[:])



            hm = wp.tile([128, DIM], BF16)
            nc.sync.dma_start(hm[:], dHM[:])
            pw = wp.tile([128, 2 * DIM], BF16)   # PW as 2 K-half tiles side by side
            nc.sync.dma_start(pw[:, 0:DIM], dPW[0:128, :])
            nc.sync.dma_start(pw[:, DIM:2*DIM], dPW[128:256, :])
            diagw = wp.tile([128, 2 * 10 * 2 * 128], FP8)
            nc.sync.dma_start(diagw[:], dDIAGW[:])

            def dw_ap(plane, tap, pt, jstride, jn=2):
                # lhsT view [128, 2, 128] into diagw
                off = plane * 2560 + tap * 256 + pt * 128
                return AP(diagw[:].tensor, off, [[5120, 128], [jstride, jn], [1, 128]])

            # ---------------- per-branch pipeline, phase-interleaved ----------------
            st = [dict(), dict()]
            qh = {}
            ps_av2 = psav.tile([128, 512], F32, tag="av")

            qnt_t = wp.tile([128, N], FP8, name="qnt")   # (h,a) x n, normalized q-attn

            def ph_qpath(g0, g1):
                qnt = qnt_t
                for grp in range(g0, g1):
                    lqg = sp.tile([128, 512], BF16, tag="lq", bufs=2, name="lqg")
                    for jj in range(4):
                        j = grp * 4 + jj
                        nc.vector.scalar_tensor_tensor(
                            lqg[:, jj*128:(jj+1)*128], u128[:], gcols_t[:, j:j+1],
                            abt[:, j*128:(j+1)*128], AL.mult, AL.add)
                        if flags["has_qb"]:
                            nc.vector.tensor_tensor(lqg[:, jj*128:(jj+1)*128],
                                                    lqg[:, jj*128:(jj+1)*128], r128[:], AL.add)
                    nc.scalar.activation(lqg[:], lqg[:], AF.Exp)
                    sqg = sp.tile([128, 32], F32, tag="sq", bufs=2, name="sqg")
                    nc.vector.tensor_reduce(sqg[:], lqg[:].rearrange("p (g b) -> p g b", b=16),
                                            mybir.AxisListType.X, AL.add)
                    rqg = sp.tile([128, 32], F32, tag="rq", bufs=2, name="rqg")
                    nc.vector.reciprocal(rqg[:], sqg[:])
                    qng = lqg
                    nc.vector.tensor_tensor(
                        qng[:].rearrange("p (g b) -> p g b", b=16),
                        qng[:].rearrange("p (g b) -> p g b", b=16),
                        rqg[:].unsqueeze(2).broadcast_to([128, 32, 16]), AL.mult)
                    for half in range(2):
                        ps_q = pssm.tile([128, 256], BF16, tag="sm", name="ps_q")
                        for q2 in range(2):
                            jj = half * 2 + q2
                            nc.tensor.transpose(ps_q[:, q2*128:(q2+1)*128],
                                                qng[:, jj*128:(jj+1)*128], ident)
                        base = (grp * 4 + half * 2) * 128
                        nc.vector.tensor_copy(qnt[:, base:base+256], ps_q[:])
                qh["qnt"] = qnt

            rhsb = wp.tile([32, N], FP8)
            nc.sync.dma_start(rhsb[:], dRHSB[:])
            rhsas = []
            for m in range(2):
                rhsa_m = bigp.tile([128, N], FP8, tag="rhsa", name=f"rhsa{m}", bufs=2)
                rhsas.append(rhsa_m)
                nc.sync.dma_start(rhsa_m[16:128, :], dRHSA[:])

            def ph_vt_kwf(m):
                # v8: fp8 hi/lo planes with zero-padded y borders.  The
                # psum is evacuated once to bf16 (ACT); the fp8 hi copy and
                # the deferred lo = v - hi both run on Pool (SBUF-only).
                v8 = [bigp.tile([128, 2 * VPLANE], FP8, tag=f"v8{pt}", name=f"v8{m}{pt}", bufs=2)
                      for pt in range(2)]
                vbf = [bigp.tile([128, N], BF16, tag=f"pre{pt}", name=f"vbf{m}{pt}", bufs=2)
                       for pt in range(2)]
                for pt in range(2):
                    for plane in range(2):
                        nc.gpsimd.memset(v8[pt][:, plane*VPLANE:plane*VPLANE + VPAD], 0.0)
                        nc.gpsimd.memset(v8[pt][:, plane*VPLANE + VPAD + N:(plane+1)*VPLANE], 0.0)
                rhsa = rhsas[m]
                for t in range(NT):
                    for pt in range(2):
                        ps_v = psb.tile([128, NTW], F32, tag="big")
                        for kh in range(2):
                            nc.tensor.matmul(
                                ps_v[:],
                                wvmm[:, kh*DIM + pt*128: kh*DIM + (pt+1)*128],
                                xts[m][kh][:, t*NTW:(t+1)*NTW],
                                start=(kh == 0), stop=(kh == 1))
                        hi = v8[pt][:, VPAD + t*NTW: VPAD + (t+1)*NTW]
                        vb = vbf[pt][:, t*NTW:(t+1)*NTW]
                        if flags["has_kvb_v"]:
                            nc.vector.tensor_scalar(vb, ps_v[:], bvcol[:, pt:pt+1], None, AL.add)
                        elif m == 1 or t % 2 == 0:
                            nc.scalar.copy(vb, ps_v[:])
                        else:
                            nc.vector.tensor_copy(vb, ps_v[:])
                        nc.gpsimd.tensor_copy(hi, vb)
                    ps_k = psh.tile([16, NTW], F32, tag="half")
                    for kh in range(2):
                        nc.tensor.matmul(
                            ps_k[:], wvmm[:, 512 + kh*16: 512 + (kh+1)*16],
                            xts[m][kh][:, t*NTW:(t+1)*NTW],
                            start=(kh == 0), stop=(kh == 1))
                    if t % 2 == 0:
                        nc.vector.tensor_copy(rhsa[0:16, t*NTW:(t+1)*NTW], ps_k[:])
                    else:
                        nc.scalar.copy(rhsa[0:16, t*NTW:(t+1)*NTW], ps_k[:])
                st[m]["v8"] = v8
                st[m]["vbf"] = vbf

            def ph_vlo(m):
                # deferred lo-plane: Pool computes v - hi in the background
                v8, vbf = st[m]["v8"], st[m]["vbf"]
                for pt in range(2):
                    for t in range(NT):
                        lo = v8[pt][:, VPLANE + VPAD + t*NTW: VPLANE + VPAD + (t+1)*NTW]
                        hi = v8[pt][:, VPAD + t*NTW: VPAD + (t+1)*NTW]
                        nc.gpsimd.tensor_tensor(lo, vbf[pt][:, t*NTW:(t+1)*NTW], hi,
                                                AL.subtract)

            def logits_alloc(m):
                st[m]["attn"] = bigp.tile([128, N], FP8, tag="attn", name=f"attn{m}", bufs=2)
                st[m]["s1p"] = bigp.tile([128, NT], F32, tag="s1p", name=f"s1p{m}", bufs=2)

            def logits_strip(m, t):
                rhsa, attn, s1p = rhsas[m], st[m]["attn"], st[m]["s1p"]
                ps_l = psb.tile([128, NTW], F32, tag="big")
                nc.tensor.matmul(ps_l[:], loga[:], rhsa[:, t*NTW:(t+1)*NTW],
                                 start=True, stop=False)
                nc.tensor.matmul(ps_l[:], logb[:], rhsb[:, t*NTW:(t+1)*NTW],
                                 start=False, stop=True)
                nc.scalar.activation(attn[:, t*NTW:(t+1)*NTW], ps_l[:], AF.Exp,
                                     accum_out=s1p[:, t:t+1])

            def logits_fin(m):
                s1p = st[m]["s1p"]
                s1 = bigp.tile([128, 1], F32, tag="s1", name=f"s1{m}", bufs=2)
                nc.vector.tensor_reduce(s1[:], s1p[:], mybir.AxisListType.X, AL.add)
                rs1 = bigp.tile([128, 1], F32, tag="rs1", name=f"rs1{m}", bufs=2)
                nc.vector.reciprocal(rs1[:], s1[:])
                st[m]["rs1"] = rs1

            def ph_logits(m):
                logits_alloc(m)
                for t in range(NT):
                    logits_strip(m, t)
                logits_fin(m)

            def ph_transp_av(m, filler=None):
                attn, v8, rs1 = st[m]["attn"], st[m]["v8"], st[m]["rs1"]
                ps_av = ps_av2[:, m*DIM:(m+1)*DIM]
                for kp in range(NCH // 2):
                    if filler is not None:
                        filler(kp)
                    # fp8 transpose must write element-step-2 psum; layout per
                    # q (stride-2 slots): [aT(256) | vT0(256) | vT1(256)]
                    ps_t = psh.tile([128, 1536], FP8, tag="half")
                    for q in range(2):
                        k = kp * 2 + q
                        nc.tensor.transpose(
                            AP(ps_t[:].tensor, q*768, [[1536, 128], [2, 128]]),
                            attn[:, k*128:(k+1)*128], ident8[:])
                        for pt in range(2):
                            nc.tensor.transpose(
                                AP(ps_t[:].tensor, q*768 + 256 + pt*256,
                                   [[1536, 128], [2, 128]]),
                                v8[pt][:, VPAD + k*128:VPAD + (k+1)*128], ident8[:])
                    tv = sp.tile([128, 768], FP8, tag="tv", name=f"tv{m}", bufs=2)
                    src_ap = AP(ps_t[:].tensor, 0, [[1536, 128], [2, 768]])
                    if kp % 2 == 0:
                        nc.vector.tensor_copy(tv[:], src_ap)
                    else:
                        nc.scalar.copy(tv[:], src_ap)
                    # one DR op: j packs the two q-chunks
                    lhs = AP(tv[:].tensor, 0, [[768, 128], [384, 2], [1, 128]])
                    rhs = AP(tv[:].tensor, 128, [[768, 128], [384, 2], [1, 256]])
                    nc.tensor.matmul(ps_av[:], lhs, rhs,
                                     start=(kp == 0), stop=(kp == NCH // 2 - 1),
                                     perf_mode=DRM)
                avf = sp.tile([128, DIM], BF16, tag="avf", name=f"avf{m}", bufs=2)
                nc.vector.scalar_tensor_tensor(avf[:], ps_av[:], rs1[:], hm[:],
                                               AL.mult, AL.mult)
                avsel = bigp.tile([128, 2 * DIM], FP8, tag="avsel", name=f"avsel{m}", bufs=2)
                nc.vector.tensor_copy(avsel[:, 0:DIM], avf[:])
                nc.vector.scalar_tensor_tensor(avsel[:, DIM:2*DIM], avsel[:, 0:DIM], -1.0,
                                               avf[:], AL.mult, AL.add)
                st[m]["avsel"] = avsel

            # (B)-op tap pairs: (0,1),(2,3),(4,5),(6,7),(8,zero)
            BPAIRS = [(0, 1), (2, 3), (4, 5), (6, 7), (8, 9)]

            def conv_alloc(m):
                st[m]["pre"] = [bigp.tile([128, N], BF16, tag=f"pre{pt}", name=f"pre{m}{pt}", bufs=2)
                                for pt in range(2)]

            def conv_strip(m, pt, t):
                v8, avsel, pre = st[m]["v8"], st[m]["avsel"], st[m]["pre"]
                qnt = qh["qnt"]
                ps_n = psb.tile([128, NTW], F32, tag="big")
                base = VPAD + t * NTW
                # (A): per tap, [w_hi | w_lo] @ [v_hi(s) | v_hi(s)]
                for k in range(9):
                    rhs = AP(v8[pt][:].tensor, base + TAPS[k],
                             [[2 * VPLANE, 128], [0, 2], [1, NTW]])
                    nc.tensor.matmul(ps_n[:], dw_ap(0, k, pt, 2560), rhs,
                                     start=(k == 0), stop=False,
                                     perf_mode=DRM, skip_group_check=True)
                # (B): tap pairs, [w_hi(A) | w_hi(B)] @ [v_lo(sA) | v_lo(sB)]
                for ka, kb in BPAIRS:
                    sa = TAPS[ka]
                    sb = TAPS[kb] if kb < 9 else sa
                    rhs = AP(v8[pt][:].tensor, VPLANE + base + sa,
                             [[2 * VPLANE, 128], [sb - sa, 2], [1, NTW]])
                    nc.tensor.matmul(ps_n[:], dw_ap(0, ka, pt, 256), rhs,
                                     start=False, stop=False,
                                     perf_mode=DRM, skip_group_check=True)
                # attn output: [avsel_hi | avsel_lo] @ [qnt | qnt]
                lhs = AP(avsel[:].tensor, pt * 128, [[512, 128], [256, 2], [1, 128]])
                rhs = AP(qnt[:].tensor, t * NTW, [[N, 128], [0, 2], [1, NTW]])
                nc.tensor.matmul(ps_n[:], lhs, rhs,
                                 start=False, stop=True,
                                 perf_mode=DRM, skip_group_check=True)
                sl = pre[pt][:, t*NTW:(t+1)*NTW]
                if flags["has_dwcb"]:
                    nc.vector.tensor_scalar(sl, ps_n[:],
                                            dwbcol[:, pt:pt+1], None, AL.add)
                elif t % 2 == 0:
                    nc.scalar.copy(sl, ps_n[:])
                else:
                    nc.vector.tensor_copy(sl, ps_n[:])

            # x-wraparound border corrections, batched over the full image.
            # With the zero-padded planes the flat-shift taps run unclipped,
            # so every row whose wrapped read lands on real data needs a fix:
            # col 63 of pre[y] wrongly got w_k * v[y+dy+1, 0] (dx=+1 taps),
            # col 0 wrongly got w_k * v[y+dy-1, 63] (dx=-1 taps).
            CORR_HI = ((2, 0, 0, 64), (5, 1, 0, 63), (8, 2, 0, 62))   # k, off, ya, yb
            CORR_LO = ((0, -2, 2, 64), (3, -1, 1, 64), (6, 0, 0, 64))

            def conv_corr_strip(m, pt, t):
                # strip-local x-wrap corrections (rows t*8 .. t*8+8)
                v8, pre = st[m]["v8"], st[m]["pre"]
                pre3 = pre[pt][:].rearrange("p (y x) -> p y x", y=H)
                r0, r1 = t * ROWS_PER_NT, (t + 1) * ROWS_PER_NT
                for xe, corr in ((63, CORR_HI), (0, CORR_LO)):
                    for k, off, ya, yb in corr:
                        ya2, yb2 = max(ya, r0), min(yb, r1)
                        if yb2 <= ya2:
                            continue
                        for plane in range(2):
                            nc.vector.scalar_tensor_tensor(
                                pre3[:, ya2:yb2, xe:xe+1],
                                AP(v8[pt][:].tensor,
                                   plane * VPLANE + VPAD + (ya2 + off) * W + (63 - xe),
                                   [[2 * VPLANE, 128], [W, yb2 - ya2], [1, 1]]),
                                neg9[:, pt*9 + k:pt*9 + k + 1],
                                pre3[:, ya2:yb2, xe:xe+1],
                                AL.mult, AL.add)

            def conv_corr(m, pt):
                v8, pre = st[m]["v8"], st[m]["pre"]
                pre3 = pre[pt][:].rearrange("p (y x) -> p y x", y=H)
                for xe, corr in ((63, CORR_HI), (0, CORR_LO)):
                    for k, off, ya, yb in corr:
                        for plane in range(2):
                            nc.vector.scalar_tensor_tensor(
                                pre3[:, ya:yb, xe:xe+1],
                                AP(v8[pt][:].tensor,
                                   plane * VPLANE + VPAD + (ya + off) * W + (63 - xe),
                                   [[2 * VPLANE, 128], [W, yb - ya], [1, 1]]),
                                neg9[:, pt*9 + k:pt*9 + k + 1],
                                pre3[:, ya:yb, xe:xe+1],
                                AL.mult, AL.add)

            def ph_tail_proj_tile(m, t):
                pre, xt = st[m]["pre"], xts[m]
                for mt in range(2):
                    ps_o = psh.tile([128, NTW], F32, tag="half")
                    fold_resid = (m == 1 and not flags["has_projb"])
                    for kh in range(2):
                        nc.tensor.matmul(
                            ps_o[:], pw[:, kh*DIM + mt*128: kh*DIM + (mt+1)*128],
                            pre[kh][:, t*NTW:(t+1)*NTW],
                            start=(kh == 0), stop=(kh == 1 and not fold_resid))
                    ot = sp.tile([128, NTW], BF16, tag="ot", name=f"ot{m}", bufs=4)
                    if fold_resid:
                        nc.tensor.matmul(ps_o[:], idf[:], xt[mt][:, t*NTW:(t+1)*NTW],
                                         start=False, stop=True)
                        nc.scalar.copy(ot[:], ps_o[:])
                    else:
                        nc.vector.scalar_tensor_tensor(
                            ot[:], ps_o[:], projb[:, mt:mt+1],
                            xt[mt][:, t*NTW:(t+1)*NTW], AL.add, AL.add)
                    nc.sync.dma_start(o_out[m][mt*128:(mt+1)*128, t*NTW:(t+1)*NTW], ot[:])

            ph_vt_kwf(0)
            ph_qpath(0, 8)
            ph_vt_kwf(1)
            ph_logits(0)
            ph_vlo(0)
            logits_alloc(1)

            def fill_log1(kp):
                if kp % 2 == 0 and kp // 2 < NT:
                    logits_strip(1, kp // 2)
            ph_transp_av(0, filler=fill_log1)
            logits_fin(1)
            ph_vlo(1)
            conv_alloc(0)

            def fill_conv0(kp):
                pt, t = divmod(kp, NT)
                conv_strip(0, pt, t)
            ph_transp_av(1, filler=fill_conv0)
            for pt in range(2):
                conv_corr(0, pt)
            conv_alloc(1)
            # interleave branch-1 conv with branch-0 proj; then pipeline
            # branch-1 proj into branch-1/pt-1 conv via strip-local
            # corrections
            for t in range(NT):
                conv_strip(1, 0, t)
                ph_tail_proj_tile(0, t)
            conv_corr(1, 0)
            for t in range(NT):
                conv_strip(1, 1, t)
                conv_corr_strip(1, 1, t)
                if t >= 1:
                    ph_tail_proj_tile(1, t - 1)
            for t in range(NT - 1, NT):
                ph_tail_proj_tile(1, t)

    nc.compile()
    return nc


# ----------------------------------------------------------------------------
# public entry point
# ----------------------------------------------------------------------------

_CACHE = {}


def kernel(**inputs):
    inputs = {k: np.asarray(v) for k, v in inputs.items()}
    params, flags = _host_precompute(
        **{k: inputs[k] for k in
           ("kv_w", "kv_b", "q_w", "q_b", "proj_w", "proj_b", "dwc_w", "dwc_b",
            "an_bias", "na_bias", "ah_bias", "aw_bias", "ha_bias", "wa_bias")})

    key = tuple(sorted(flags.items()))
    if key not in _CACHE:
        _CACHE[key] = _build(flags)
    nc = _CACHE[key]

    in_maps = _make_in_maps(inputs, params)

    res = run_bass_kernel_spmd(nc, in_maps, core_ids=list(range(B)))
    o1 = np.stack([res.results[b]["o1"].reshape(DIM, H, W) for b in range(B)])
    o2 = np.stack([res.results[b]["o2"].reshape(DIM, H, W) for b in range(B)])
    return o1.astype(np.float32), o2.astype(np.float32)


def _make_in_maps(inputs, params):
    input1, input2, guidmap = inputs["input1"], inputs["input2"], inputs["guidmap"]
    qmeta = params["qmeta"]
    shared = {
        "LOGC_A": params["LOGC_A"], "LOGC_B": params["LOGC_B"],
        "RHSC_A": params["RHSC_A"], "RHSC_B": params["RHSC_B"],
        "ABt": params["ABt"], "WVMM": params["WVMM"], "PW": params["PW"],
        "DIAGW": np.ascontiguousarray(params["DIAGW"].reshape(128, -1)),
        "HM": params["HM"], "IDENT8": params["IDENT8"], "IDF32": params["IDF32"],
        "SMALL_BF": params["SMALL_BF"], "SMALL_F32": params["SMALL_F32"],
    }
    in_maps = []
    for b in range(B):
        g = guidmap[b].reshape(N).astype(np.float32)
        gimg = g.reshape(H, W)
        gblk = gimg.reshape(PS, AGENT, PS, AGENT).transpose(0, 2, 1, 3).reshape(AGENT, 256)
        gcols = g.reshape(NCH, 128).T.copy()
        # agent means in (h,a)-expanded order: gbar128[a + 16*h-ish] follows
        # BLK expansion: gbar128[ha] = gbar[ha % 16]
        gbar = gblk.mean(axis=1)                       # (16,)
        gbar128 = np.tile(gbar, HEADS).reshape(HEADS, AGENT)
        gbar128 = gbar.reshape(1, 16)
        gb = np.zeros(128, np.float32)
        for a in range(16):
            gb[a::16] = gbar[a]
        EG8 = (qmeta["EgC"] * gb[None, :]).astype(F8)   # (8,128)
        qr = qmeta["qrows"]
        u_row = gb * qr[0]
        r_row = None
        if np.any(qr[1]) or np.any(qr[2]):
            u_row = u_row + qr[1]
            r_row = gb * qr[1] + qr[2]
        U128 = np.broadcast_to(u_row.astype(BF), (128, 128))
        R128 = (np.broadcast_to(r_row.astype(BF), (128, 128)) if r_row is not None
                else np.zeros((128, 128), BF))
        in_maps.append({
            "x1": np.ascontiguousarray(input1[b].reshape(DIM, N).astype(BF)),
            "x2": np.ascontiguousarray(input2[b].reshape(DIM, N).astype(BF)),
            "gcols": np.ascontiguousarray(gcols.astype(np.float32)),
            "EG8": np.ascontiguousarray(EG8),
            "U128": np.ascontiguousarray(U128),
            "R128": np.ascontiguousarray(R128),
            **shared,
        })
    return in_maps



# revision 9
# speedup vs baseline: 1.0262x; 1.0262x over previous
"""Trainium2 Bass kernel for nn_Cross_AgentAttention.

Data-parallel over batch B=8 across 8 NeuronCores; params replicated.

Per-core algorithm (feature-major (c, n) layout, exploiting that
q = guidmap @ q_w + q_b is rank-1):
  - v = x @ Wv via fp32r matmuls (TF32-grade, full PE rate), evacuated as
    fp8 hi+lo planes (hi = fp8(v), lo = fp8(v-hi)) with zero-padded y
    borders.
  - agent->kv attention collapses to kw[h,i] = w_h . k_h[i], computed as
    x @ Mkw (folded on host); logits = scale*gbar_a*kw[h,i] + PB, where
    row-constant terms cancel in softmax.  attn = exp(logits) stored fp8.
  - query->agent attention collapses to a rank-1 logit map
    lq[i,(h,a)] = g_i * u[(h,a)] (+r) + ABt; qnt stored fp8.
  - agent_v via fp8 DoubleRow matmuls over transposed attn/v chunks
    (both q-chunks of a pair packed in the two K-tiles of one DR op).
  - depthwise 3x3 conv: every tap is fp8 DoubleRow matmuls against the
    padded v hi/lo planes; weight fp8 residuals get their own paired DR
    ops, so the conv is bf16-grade accurate at fp8 double-pump rate.
  - x-wraparound border corrections batched once per branch.
  - attn-output matmul = one DR op per strip (avsel hi/lo planes).
"""
import numpy as np
import ml_dtypes

import concourse.bass as bass
import concourse.bacc as bacc
import concourse.mybir as mybir
from concourse.tile import TileContext
from concourse.ap import AP
from concourse.bass_utils import run_bass_kernel_spmd

F32 = mybir.dt.float32
F32R = mybir.dt.float32r
BF16 = mybir.dt.bfloat16
FP8 = mybir.dt.float8e4
BF = ml_dtypes.bfloat16
F8 = ml_dtypes.float8_e4m3
DRM = mybir.MatmulPerfMode.DoubleRow

DIM = 256
HEADS = 8
AGENT = 16
H = W = 64
B = 8
N = H * W                 # 4096
HD = DIM // HEADS         # 32
SCALE = HD ** -0.5
PS = 4
NT = 8                    # n-tiles of 512
NTW = N // NT             # 512
NCH = 32                  # n-chunks of 128
ROWS_PER_NT = NTW // W    # 8 image rows per n-tile
VPAD = 128                # zero pad on each side of each v plane
VPLANE = VPAD + N + VPAD  # 4352
# conv tap shifts, row-major (dy,dx) in (-1,0,1)^2
TAPS = [dy * W + dx for dy in (-1, 0, 1) for dx in (-1, 0, 1)]

AL = mybir.AluOpType
AF = mybir.ActivationFunctionType


# ----------------------------------------------------------------------------
# host precompute
# ----------------------------------------------------------------------------

def _bilinear_matrix(n_in, n_out):
    U = np.zeros((n_out, n_in), dtype=np.float64)
    s = n_in / n_out
    for o in range(n_out):
        x = (o + 0.5) * s - 0.5
        x0 = int(np.floor(x))
        t = x - x0
        for i, wt in ((x0, 1.0 - t), (x0 + 1, t)):
            ic = min(max(i, 0), n_in - 1)
            U[o, ic] += wt
    return U.astype(np.float32)


def _host_precompute(kv_w, kv_b, q_w, q_b, proj_w, proj_b, dwc_w, dwc_b,
                     an_bias, na_bias, ah_bias, aw_bias, ha_bias, wa_bias):
    c = DIM
    w = q_w[0]
    beta = q_b
    U = _bilinear_matrix(PS, H)

    # logits-matmul constant operands: logits = LOG^T @ RHS with
    # LOG rows = [Eg-gbar(8, device) | EgC(8) | an_tbl(16) | ahT(64) | awT(64)]
    # RHS rows = [kw(8, device) | kbeta(8, device) | UU(16) | Yind(64) | Xind(64)]
    an_tbl = an_bias.reshape(HEADS * AGENT, PS * PS).T.astype(np.float32)      # (16, 128)
    UU = np.einsum("yr,xc->rcyx", U, U).reshape(PS * PS, N).astype(np.float32)  # (16, 4096)
    ahT = ah_bias[0][..., 0].reshape(HEADS * AGENT, H).T.astype(np.float32)    # (64, 128)
    awT = aw_bias[0][:, :, 0, :].reshape(HEADS * AGENT, W).T.astype(np.float32)
    Yind = np.kron(np.eye(H, dtype=np.float32), np.ones((1, W), np.float32))    # (64, 4096)
    Xind = np.concatenate([np.eye(W, dtype=np.float32)] * H, axis=1)            # (64, 4096)

    na_up = np.einsum("yr,harc,xc->hayx", U, na_bias.reshape(HEADS, AGENT, PS, PS), U)
    ab = na_up.reshape(HEADS, AGENT, N).transpose(0, 2, 1)
    ab = ab + (ha_bias[0] + wa_bias[0]).reshape(HEADS, N, AGENT)
    ABt = ab.transpose(1, 0, 2).reshape(N, HEADS * AGENT).astype(np.float32)

    wk = kv_w[:, :c]
    Mkw = np.stack([(wk[:, h*HD:(h+1)*HD] * w[None, h*HD:(h+1)*HD]).sum(1)
                    for h in range(HEADS)], axis=1)
    Mkb = np.stack([(wk[:, h*HD:(h+1)*HD] * beta[None, h*HD:(h+1)*HD]).sum(1)
                    for h in range(HEADS)], axis=1)
    MM = np.concatenate([Mkw, Mkb], axis=1).astype(np.float32)      # (256, 16)

    hw2 = np.array([(w[h*HD:(h+1)*HD]**2).sum() for h in range(HEADS)], np.float32)
    wb = np.array([(w[h*HD:(h+1)*HD]*beta[h*HD:(h+1)*HD]).sum() for h in range(HEADS)], np.float32)
    bb = np.array([(beta[h*HD:(h+1)*HD]**2).sum() for h in range(HEADS)], np.float32)
    # qrows: [s*hw2 | s*wb | s*bb] repeated per agent -> (1, 384)
    qrows = np.concatenate([np.repeat(SCALE * hw2, AGENT),
                            np.repeat(SCALE * wb, AGENT),
                            np.repeat(SCALE * bb, AGENT)])[None, :].astype(np.float32)

    Wv = kv_w[:, c:].astype(np.float32)                              # (256, 256)
    bv = kv_b[c:].astype(np.float32)

    headmask = np.zeros((HEADS * AGENT, c), np.float32)
    for h in range(HEADS):
        headmask[h*AGENT:(h+1)*AGENT, h*HD:(h+1)*HD] = 1.0

    # EgC: constant rows 8..15 of Eg (selector for the k-beta stream)
    EgC = np.zeros((8, 128), np.float32)
    for h in range(HEADS):
        EgC[h, h*AGENT:(h+1)*AGENT] = SCALE
    HB8 = EgC.copy()   # same pattern masks the gbar broadcast into Eg rows 0..7

    # DIAGW: fp8 hi/lo diagonal tap matrices.
    # layout [k=128, plane(2: hi,lo), tap(10: 0..8 + zero), pt(2), m=128]
    dwc9 = dwc_w.reshape(c, 9).astype(np.float32)
    w_hi = dwc9.astype(F8).astype(np.float32)
    w_lo = (dwc9 - w_hi).astype(F8).astype(np.float32)
    DIAGW = np.zeros((128, 2, 10, 2, 128), np.float32)
    for plane, wsrc in ((0, w_hi), (1, w_lo)):
        for t in range(9):
            for pt in range(2):
                np.fill_diagonal(DIAGW[:, plane, t, pt, :], wsrc[pt*128:(pt+1)*128, t])

    BLK = np.zeros((16, 128), np.float32)                            # gbar -> (h,a) expand
    for a in range(16):
        BLK[a, a::16] = 1.0

    NEG9 = np.zeros((128, 18), np.float32)
    for pt in range(2):
        NEG9[:, pt*9:(pt+1)*9] = -dwc9[pt*128:(pt+1)*128, :]

    projb = np.stack([proj_b[:128], proj_b[128:]], axis=1).astype(np.float32)  # (128, 2)
    bvcol = np.stack([bv[:128], bv[128:]], axis=1).astype(np.float32)          # (128, 2)
    dwbcol = np.stack([dwc_b[:128], dwc_b[128:]], axis=1).astype(np.float32)   # (128, 2)

    flags = dict(
        has_qb=bool(np.any(q_b != 0)),
        has_kvb_v=bool(np.any(bv != 0)),
        has_dwcb=bool(np.any(dwc_b != 0)),
        has_projb=bool(np.any(proj_b != 0)),
    )
    qmeta = dict(qrows=np.concatenate([np.repeat(SCALE * hw2, AGENT),
                                       np.repeat(SCALE * wb, AGENT),
                                       np.repeat(SCALE * bb, AGENT)]).reshape(3, 128),
                 EgC=EgC)

    LOGC_A = np.zeros((128, 128), np.float32)
    LOGC_A[8:16] = EgC
    LOGC_A[16:32] = an_tbl
    LOGC_A[32:96] = ahT
    LOGC_A[96:128] = awT[0:32]
    LOGC_B = np.zeros((128, 128), np.float32)        # plane-1 lhsT, zero-padded
    LOGC_B[0:32] = awT[32:64]
    RHSC_A = np.concatenate([UU, Yind, Xind[0:32]], axis=0)   # (112, 4096)
    RHSC_B = np.zeros((128, N), np.float32)                   # zero-padded plane B
    RHSC_B[0:32] = Xind[32:64]

    # SMALL_BF (128, 144): [ident 0:128 | i16 128:144]
    SMALL_BF = np.zeros((128, 144), np.float32)
    SMALL_BF[:, 0:128] = np.eye(128, dtype=np.float32)
    SMALL_BF[0:16, 128:144] = np.eye(16, dtype=np.float32)
    # SMALL_F32 (128, 24): [neg9 0:18 | projb 18:20 | bvcol 20:22 | dwb 22:24]
    SMALL_F32 = np.concatenate([NEG9, projb, bvcol, dwbcol], axis=1)

    IDENT8 = np.eye(128, dtype=np.float32)
    IDF32 = np.eye(128, dtype=np.float32)

    params = dict(
        LOGC_A=LOGC_A.astype(F8), LOGC_B=LOGC_B.astype(F8),
        RHSC_A=RHSC_A.astype(F8), RHSC_B=RHSC_B.astype(F8),
        ABt=ABt.astype(F8),
        WVMM=np.concatenate([Wv[0:128], Wv[128:256], MM[0:128], MM[128:256]],
                            axis=1).astype(BF),
        PW=proj_w.astype(np.float32).astype(BF),
        DIAGW=DIAGW.astype(F8), HM=headmask.astype(BF),
        IDENT8=IDENT8.astype(F8), IDF32=IDF32.astype(BF),
        SMALL_BF=SMALL_BF.astype(BF), SMALL_F32=SMALL_F32.astype(np.float32),
    )
    params["qmeta"] = qmeta
    return params, flags


# ----------------------------------------------------------------------------
# device kernel builder
# ----------------------------------------------------------------------------

def _build(flags):
    nc = bacc.Bacc(None, target_bir_lowering=False, debug=False)

    # ---- DRAM I/O ----
    x_in = [nc.dram_tensor(f"x{m+1}", [DIM, N], BF16, kind="ExternalInput") for m in range(2)]
    gcols = nc.dram_tensor("gcols", [128, NCH], F32, kind="ExternalInput")
    dEG8 = nc.dram_tensor("EG8", [8, 128], FP8, kind="ExternalInput")
    dU128 = nc.dram_tensor("U128", [128, 128], BF16, kind="ExternalInput")
    dR128 = nc.dram_tensor("R128", [128, 128], BF16, kind="ExternalInput")
    dLOGA = nc.dram_tensor("LOGC_A", [128, 128], FP8, kind="ExternalInput")
    dLOGB = nc.dram_tensor("LOGC_B", [128, 128], FP8, kind="ExternalInput")
    dRHSA = nc.dram_tensor("RHSC_A", [112, N], FP8, kind="ExternalInput")
    dRHSB = nc.dram_tensor("RHSC_B", [128, N], FP8, kind="ExternalInput")
    dABt = nc.dram_tensor("ABt", [N, 128], FP8, kind="ExternalInput")
    dWVMM = nc.dram_tensor("WVMM", [128, 544], BF16, kind="ExternalInput")
    dPW = nc.dram_tensor("PW", [DIM, DIM], BF16, kind="ExternalInput")
    dDIAGW = nc.dram_tensor("DIAGW", [128, 2 * 10 * 2 * 128], FP8, kind="ExternalInput")
    dHM = nc.dram_tensor("HM", [128, DIM], BF16, kind="ExternalInput")
    dI8 = nc.dram_tensor("IDENT8", [128, 128], FP8, kind="ExternalInput")
    dIDF = nc.dram_tensor("IDF32", [128, 128], BF16, kind="ExternalInput")
    dSBF = nc.dram_tensor("SMALL_BF", [128, 144], BF16, kind="ExternalInput")
    dSF32 = nc.dram_tensor("SMALL_F32", [128, 24], F32, kind="ExternalInput")
    o_out = [nc.dram_tensor(f"o{m+1}", [DIM, N], BF16, kind="ExternalOutput") for m in range(2)]

    with TileContext(nc) as tc:
        with (
            tc.tile_pool(name="wpool", bufs=1) as wp,          # weights/consts
            tc.tile_pool(name="big", bufs=1) as bigp,          # big per-branch tensors
            tc.tile_pool(name="xpool", bufs=2) as xp,          # input prefetch
            tc.tile_pool(name="small", bufs=3) as sp,          # rotating small tiles
            tc.tile_pool(name="ps_big", bufs=3, space="PSUM") as psb,    # (128,512)
            tc.tile_pool(name="ps_half", bufs=3, space="PSUM") as psh,   # (128,256)
            tc.tile_pool(name="ps_sm", bufs=1, space="PSUM") as pssm,    # (128,128)
            tc.tile_pool(name="ps_av", bufs=1, space="PSUM") as psav,
        ):
            # ---------------- critical-path DMAs first ----------------
            # tiny q-path seeds first (they head the DVE queue), then the
            # v-matmul operands strip-chunked so the first matmul starts early
            wvmm = wp.tile([128, 544], BF16)
            nc.sync.dma_start(wvmm[:], dWVMM[:])
            wv_f = wvmm[:, 0:512]

            xts = []
            smallbf = wp.tile([128, 144], BF16)
            gcols_t = wp.tile([128, NCH], F32)
            smallf = wp.tile([128, 24], F32)
            for m in range(2):
                xtm = [xp.tile([128, N], BF16, tag=f"x{m}{pt}", name=f"xt{m}{pt}", bufs=1) for pt in range(2)]
                if m == 0:
                    for t2 in range(NT // 2):
                        for pt in range(2):
                            nc.sync.dma_start(xtm[pt][:, t2*2*NTW:(t2+1)*2*NTW],
                                              x_in[0][pt*128:(pt+1)*128, t2*2*NTW:(t2+1)*2*NTW])
                    nc.sync.dma_start(smallbf[:], dSBF[:])
                    nc.sync.dma_start(gcols_t[:], gcols[:])
                    nc.sync.dma_start(smallf[:], dSF32[:])
                xts.append(xtm)

            mm_f = wvmm[:, 512:544]

            u128 = wp.tile([128, 128], BF16)
            nc.sync.dma_start(u128[:], dU128[:])
            logab = wp.tile([128, 256], FP8)
            nc.sync.dma_start(logab[8:128, 0:128], dLOGA[8:128, :])
            nc.sync.dma_start(logab[0:8, 0:128], dEG8[:])
            nc.sync.dma_start(logab[:, 128:256], dLOGB[:])
            if flags["has_qb"]:
                r128 = wp.tile([128, 128], BF16)
                nc.sync.dma_start(r128[:], dR128[:])

            abt = bigp.tile([128, NCH * 128], FP8, tag="attn", bufs=2)
            nc.sync.dma_start(
                abt[:].rearrange("p (j f) -> p j f", j=NCH),
                dABt[:].rearrange("(j p) f -> p j f", j=NCH))

            for t2 in range(NT // 2):
                for pt in range(2):
                    nc.sync.dma_start(xts[1][pt][:, t2*2*NTW:(t2+1)*2*NTW],
                                      x_in[1][pt*128:(pt+1)*128, t2*2*NTW:(t2+1)*2*NTW])
            ident = smallbf[:, 0:128]
            neg9 = smallf[:, 0:18]
            projb = smallf[:, 18:20]
            bvcol = smallf[:, 20:22]
            dwbcol = smallf[:, 22:24]

            ident8 = wp.tile([128, 128], FP8)
            nc.sync.dma_start(ident8[:], dI8[:])
            idf = wp.tile([128, 128], BF16)
            nc.sync.dma_start(idf[:], dIDF[:])



            hm = wp.tile([128, DIM], BF16)
            nc.sync.dma_start(hm[:], dHM[:])
            pw = wp.tile([128, 2 * DIM], BF16)   # PW as 2 K-half tiles side by side
            nc.sync.dma_start(pw[:, 0:DIM], dPW[0:128, :])
            nc.sync.dma_start(pw[:, DIM:2*DIM], dPW[128:256, :])
            diagw = wp.tile([128, 2 * 10 * 2 * 128], FP8)
            nc.sync.dma_start(diagw[:], dDIAGW[:])

            def dw_ap(plane, tap, pt, jstride, jn=2):
                # lhsT view [128, 2, 128] into diagw
                off = plane * 2560 + tap * 256 + pt * 128
                return AP(diagw[:].tensor, off, [[5120, 128], [jstride, jn], [1, 128]])

            # ---------------- per-branch pipeline, phase-interleaved ----------------
            st = [dict(), dict()]
            qh = {}
            ps_av2 = psav.tile([128, 512], F32, tag="av")

            qnt_t = wp.tile([128, N], FP8, name="qnt")   # (h,a) x n, normalized q-attn

            def ph_qpath(g0, g1):
                qnt = qnt_t
                for grp in range(g0, g1):
                    lqg = sp.tile([128, 512], BF16, tag="lq", bufs=2, name="lqg")
                    for jj in range(4):
                        j = grp * 4 + jj
                        nc.vector.scalar_tensor_tensor(
                            lqg[:, jj*128:(jj+1)*128], u128[:], gcols_t[:, j:j+1],
                            abt[:, j*128:(j+1)*128], AL.mult, AL.add)
                        if flags["has_qb"]:
                            nc.vector.tensor_tensor(lqg[:, jj*128:(jj+1)*128],
                                                    lqg[:, jj*128:(jj+1)*128], r128[:], AL.add)
                    nc.scalar.activation(lqg[:], lqg[:], AF.Exp)
                    sqg = sp.tile([128, 32], F32, tag="sq", bufs=2, name="sqg")
                    nc.vector.tensor_reduce(sqg[:], lqg[:].rearrange("p (g b) -> p g b", b=16),
                                            mybir.AxisListType.X, AL.add)
                    rqg = sp.tile([128, 32], F32, tag="rq", bufs=2, name="rqg")
                    nc.vector.reciprocal(rqg[:], sqg[:])
                    qng = lqg
                    nc.vector.tensor_tensor(
                        qng[:].rearrange("p (g b) -> p g b", b=16),
                        qng[:].rearrange("p (g b) -> p g b", b=16),
                        rqg[:].unsqueeze(2).broadcast_to([128, 32, 16]), AL.mult)
                    for half in range(2):
                        ps_q = pssm.tile([128, 256], BF16, tag="sm", name="ps_q")
                        for q2 in range(2):
                            jj = half * 2 + q2
                            nc.tensor.transpose(ps_q[:, q2*128:(q2+1)*128],
                                                qng[:, jj*128:(jj+1)*128], ident)
                        base = (grp * 4 + half * 2) * 128
                        nc.vector.tensor_copy(qnt[:, base:base+256], ps_q[:])
                qh["qnt"] = qnt

            rhs2s = []
            for m in range(2):
                rhs2_m = bigp.tile([128, 2 * N], FP8, tag="rhsa", name=f"rhsa{m}", bufs=2)
                rhs2s.append(rhs2_m)
                nc.sync.dma_start(rhs2_m[16:128, 0:N], dRHSA[:])
                nc.sync.dma_start(rhs2_m[:, N:2 * N], dRHSB[:])

            def ph_vt_kwf(m):
                # v8: fp8 hi/lo planes with zero-padded y borders.  The
                # psum is evacuated once to bf16 (ACT); the fp8 hi copy and
                # the deferred lo = v - hi both run on Pool (SBUF-only).
                v8 = [bigp.tile([128, 2 * VPLANE], FP8, tag=f"v8{pt}", name=f"v8{m}{pt}", bufs=2)
                      for pt in range(2)]
                vbf = [bigp.tile([128, N], BF16, tag=f"pre{pt}", name=f"vbf{m}{pt}", bufs=2)
                       for pt in range(2)]
                for pt in range(2):
                    for plane in range(2):
                        nc.gpsimd.memset(v8[pt][:, plane*VPLANE:plane*VPLANE + VPAD], 0.0)
                        nc.gpsimd.memset(v8[pt][:, plane*VPLANE + VPAD + N:(plane+1)*VPLANE], 0.0)
                rhsa = rhs2s[m]
                for t in range(NT):
                    for pt in range(2):
                        ps_v = psb.tile([128, NTW], F32, tag="big")
                        for kh in range(2):
                            nc.tensor.matmul(
                                ps_v[:],
                                wvmm[:, kh*DIM + pt*128: kh*DIM + (pt+1)*128],
                                xts[m][kh][:, t*NTW:(t+1)*NTW],
                                start=(kh == 0), stop=(kh == 1))
                        hi = v8[pt][:, VPAD + t*NTW: VPAD + (t+1)*NTW]
                        vb = vbf[pt][:, t*NTW:(t+1)*NTW]
                        if flags["has_kvb_v"]:
                            nc.vector.tensor_scalar(vb, ps_v[:], bvcol[:, pt:pt+1], None, AL.add)
                        elif m == 1 or t % 2 == 0:
                            nc.scalar.copy(vb, ps_v[:])
                        else:
                            nc.vector.tensor_copy(vb, ps_v[:])
                        nc.gpsimd.tensor_copy(hi, vb)
                    ps_k = psh.tile([16, NTW], F32, tag="half")
                    for kh in range(2):
                        nc.tensor.matmul(
                            ps_k[:], wvmm[:, 512 + kh*16: 512 + (kh+1)*16],
                            xts[m][kh][:, t*NTW:(t+1)*NTW],
                            start=(kh == 0), stop=(kh == 1))
                    if t % 2 == 0:
                        nc.vector.tensor_copy(rhsa[0:16, t*NTW:(t+1)*NTW], ps_k[:])
                    else:
                        nc.scalar.copy(rhsa[0:16, t*NTW:(t+1)*NTW], ps_k[:])
                st[m]["v8"] = v8
                st[m]["vbf"] = vbf

            def ph_vlo(m):
                # deferred lo-plane: Pool computes v - hi in the background
                v8, vbf = st[m]["v8"], st[m]["vbf"]
                for pt in range(2):
                    for t in range(NT):
                        lo = v8[pt][:, VPLANE + VPAD + t*NTW: VPLANE + VPAD + (t+1)*NTW]
                        hi = v8[pt][:, VPAD + t*NTW: VPAD + (t+1)*NTW]
                        nc.gpsimd.tensor_tensor(lo, vbf[pt][:, t*NTW:(t+1)*NTW], hi,
                                                AL.subtract)

            def logits_alloc(m):
                st[m]["attn"] = bigp.tile([128, N], FP8, tag="attn", name=f"attn{m}", bufs=2)
                st[m]["s1p"] = bigp.tile([128, NT], F32, tag="s1p", name=f"s1p{m}", bufs=2)

            def logits_strip(m, t):
                rhs2, attn, s1p = rhs2s[m], st[m]["attn"], st[m]["s1p"]
                ps_l = psb.tile([128, NTW], F32, tag="big")
                # one DR op: j packs the [loga | logb] planes
                lhs = AP(logab[:].tensor, 0, [[256, 128], [128, 2], [1, 128]])
                rhs = AP(rhs2[:].tensor, t * NTW, [[2 * N, 128], [N, 2], [1, NTW]])
                nc.tensor.matmul(ps_l[:], lhs, rhs, start=True, stop=True,
                                 perf_mode=DRM)
                nc.scalar.activation(attn[:, t*NTW:(t+1)*NTW], ps_l[:], AF.Exp,
                                     accum_out=s1p[:, t:t+1])

            def logits_fin(m):
                s1p = st[m]["s1p"]
                s1 = bigp.tile([128, 1], F32, tag="s1", name=f"s1{m}", bufs=2)
                nc.vector.tensor_reduce(s1[:], s1p[:], mybir.AxisListType.X, AL.add)
                rs1 = bigp.tile([128, 1], F32, tag="rs1", name=f"rs1{m}", bufs=2)
                nc.vector.reciprocal(rs1[:], s1[:])
                st[m]["rs1"] = rs1

            def ph_logits(m):
                logits_alloc(m)
                for t in range(NT):
                    logits_strip(m, t)
                logits_fin(m)

            def ph_transp_av(m, filler=None):
                attn, v8, rs1 = st[m]["attn"], st[m]["v8"], st[m]["rs1"]
                ps_av = ps_av2[:, m*DIM:(m+1)*DIM]
                for kp in range(NCH // 2):
                    if filler is not None:
                        filler(kp)
                    # fp8 transpose must write element-step-2 psum; layout per
                    # q (stride-2 slots): [aT(256) | vT0(256) | vT1(256)]
                    ps_t = psh.tile([128, 1536], FP8, tag="half")
                    for q in range(2):
                        k = kp * 2 + q
                        nc.tensor.transpose(
                            AP(ps_t[:].tensor, q*768, [[1536, 128], [2, 128]]),
                            attn[:, k*128:(k+1)*128], ident8[:])
                        for pt in range(2):
                            nc.tensor.transpose(
                                AP(ps_t[:].tensor, q*768 + 256 + pt*256,
                                   [[1536, 128], [2, 128]]),
                                v8[pt][:, VPAD + k*128:VPAD + (k+1)*128], ident8[:])
                    tv = sp.tile([128, 768], FP8, tag="tv", name=f"tv{m}", bufs=2)
                    src_ap = AP(ps_t[:].tensor, 0, [[1536, 128], [2, 768]])
                    if kp % 2 == 0:
                        nc.vector.tensor_copy(tv[:], src_ap)
                    else:
                        nc.scalar.copy(tv[:], src_ap)
                    # one DR op: j packs the two q-chunks
                    lhs = AP(tv[:].tensor, 0, [[768, 128], [384, 2], [1, 128]])
                    rhs = AP(tv[:].tensor, 128, [[768, 128], [384, 2], [1, 256]])
                    nc.tensor.matmul(ps_av[:], lhs, rhs,
                                     start=(kp == 0), stop=(kp == NCH // 2 - 1),
                                     perf_mode=DRM)
                avf = sp.tile([128, DIM], BF16, tag="avf", name=f"avf{m}", bufs=2)
                nc.vector.scalar_tensor_tensor(avf[:], ps_av[:], rs1[:], hm[:],
                                               AL.mult, AL.mult)
                avsel = bigp.tile([128, 2 * DIM], FP8, tag="avsel", name=f"avsel{m}", bufs=2)
                nc.vector.tensor_copy(avsel[:, 0:DIM], avf[:])
                nc.vector.scalar_tensor_tensor(avsel[:, DIM:2*DIM], avsel[:, 0:DIM], -1.0,
                                               avf[:], AL.mult, AL.add)
                st[m]["avsel"] = avsel

            # tap pairs for the 2-per-op DR packing
            PAIRS4 = [(0, 1), (2, 3), (4, 5), (6, 7)]

            def conv_alloc(m):
                st[m]["pre"] = [bigp.tile([128, N], BF16, tag=f"pre{pt}", name=f"pre{m}{pt}", bufs=2)
                                for pt in range(2)]

            def conv_strip(m, pt, t):
                v8, avsel, pre = st[m]["v8"], st[m]["avsel"], st[m]["pre"]
                qnt = qh["qnt"]
                ps_n = psb.tile([128, NTW], F32, tag="big")
                base = VPAD + t * NTW
                # tap pairs: plane 0 = [w_hi(A)|w_hi(B)] @ [v_hi(sA)|v_hi(sB)],
                # plane 1 = [w_lo(A)|w_lo(B)] @ same (w_lo*v_hi correction)
                first = True
                for plane in range(2):
                    for ka, kb in PAIRS4:
                        rhs = AP(v8[pt][:].tensor, base + TAPS[ka],
                                 [[2 * VPLANE, 128], [TAPS[kb] - TAPS[ka], 2],
                                  [1, NTW]])
                        nc.tensor.matmul(ps_n[:], dw_ap(plane, ka, pt, 256), rhs,
                                         start=first, stop=False,
                                         perf_mode=DRM, skip_group_check=True)
                        first = False
                # tap 8: [w_hi(8) | w_lo(8)] @ [v_hi(s8) | v_hi(s8)]
                rhs = AP(v8[pt][:].tensor, base + TAPS[8],
                         [[2 * VPLANE, 128], [0, 2], [1, NTW]])
                nc.tensor.matmul(ps_n[:], dw_ap(0, 8, pt, 2560), rhs,
                                 start=False, stop=False,
                                 perf_mode=DRM, skip_group_check=True)
                # tap pairs hi*lo: [w_hi(A)|w_hi(B)] @ [v_lo(sA)|v_lo(sB)]
                # (tap 8's hi*lo term is dropped: ~1% of one tap's magnitude)
                for ka, kb in PAIRS4:
                    rhs = AP(v8[pt][:].tensor, VPLANE + base + TAPS[ka],
                             [[2 * VPLANE, 128], [TAPS[kb] - TAPS[ka], 2],
                              [1, NTW]])
                    nc.tensor.matmul(ps_n[:], dw_ap(0, ka, pt, 256), rhs,
                                     start=False, stop=False,
                                     perf_mode=DRM, skip_group_check=True)
                # attn output: [avsel_hi | avsel_lo] @ [qnt | qnt]
                lhs = AP(avsel[:].tensor, pt * 128, [[512, 128], [256, 2], [1, 128]])
                rhs = AP(qnt[:].tensor, t * NTW, [[N, 128], [0, 2], [1, NTW]])
                nc.tensor.matmul(ps_n[:], lhs, rhs,
                                 start=False, stop=True,
                                 perf_mode=DRM, skip_group_check=True)
                sl = pre[pt][:, t*NTW:(t+1)*NTW]
                if flags["has_dwcb"]:
                    nc.vector.tensor_scalar(sl, ps_n[:],
                                            dwbcol[:, pt:pt+1], None, AL.add)
                elif t % 2 == 0:
                    nc.scalar.copy(sl, ps_n[:])
                else:
                    nc.vector.tensor_copy(sl, ps_n[:])

            # x-wraparound border corrections, batched over the full image.
            # With the zero-padded planes the flat-shift taps run unclipped,
            # so every row whose wrapped read lands on real data needs a fix:
            # col 63 of pre[y] wrongly got w_k * v[y+dy+1, 0] (dx=+1 taps),
            # col 0 wrongly got w_k * v[y+dy-1, 63] (dx=-1 taps).
            CORR_HI = ((2, 0, 0, 64), (5, 1, 0, 63), (8, 2, 0, 62))   # k, off, ya, yb
            CORR_LO = ((0, -2, 2, 64), (3, -1, 1, 64), (6, 0, 0, 64))

            def conv_corr_strip(m, pt, t):
                # strip-local x-wrap corrections (rows t*8 .. t*8+8)
                v8, pre = st[m]["v8"], st[m]["pre"]
                pre3 = pre[pt][:].rearrange("p (y x) -> p y x", y=H)
                r0, r1 = t * ROWS_PER_NT, (t + 1) * ROWS_PER_NT
                for xe, corr in ((63, CORR_HI), (0, CORR_LO)):
                    for k, off, ya, yb in corr:
                        ya2, yb2 = max(ya, r0), min(yb, r1)
                        if yb2 <= ya2:
                            continue
                        for plane in range(2):
                            nc.vector.scalar_tensor_tensor(
                                pre3[:, ya2:yb2, xe:xe+1],
                                AP(v8[pt][:].tensor,
                                   plane * VPLANE + VPAD + (ya2 + off) * W + (63 - xe),
                                   [[2 * VPLANE, 128], [W, yb2 - ya2], [1, 1]]),
                                neg9[:, pt*9 + k:pt*9 + k + 1],
                                pre3[:, ya2:yb2, xe:xe+1],
                                AL.mult, AL.add)

            def conv_corr(m, pt):
                v8, pre = st[m]["v8"], st[m]["pre"]
                pre3 = pre[pt][:].rearrange("p (y x) -> p y x", y=H)
                for xe, corr in ((63, CORR_HI), (0, CORR_LO)):
                    for k, off, ya, yb in corr:
                        for plane in range(2):
                            nc.vector.scalar_tensor_tensor(
                                pre3[:, ya:yb, xe:xe+1],
                                AP(v8[pt][:].tensor,
                                   plane * VPLANE + VPAD + (ya + off) * W + (63 - xe),
                                   [[2 * VPLANE, 128], [W, yb - ya], [1, 1]]),
                                neg9[:, pt*9 + k:pt*9 + k + 1],
                                pre3[:, ya:yb, xe:xe+1],
                                AL.mult, AL.add)

            def ph_tail_proj_tile(m, t):
                pre, xt = st[m]["pre"], xts[m]
                for mt in range(2):
                    ps_o = psh.tile([128, NTW], F32, tag="half")
                    fold_resid = (m == 1 and not flags["has_projb"])
                    for kh in range(2):
                        nc.tensor.matmul(
                            ps_o[:], pw[:, kh*DIM + mt*128: kh*DIM + (mt+1)*128],
                            pre[kh][:, t*NTW:(t+1)*NTW],
                            start=(kh == 0), stop=(kh == 1 and not fold_resid))
                    ot = sp.tile([128, NTW], BF16, tag="ot", name=f"ot{m}", bufs=4)
                    if fold_resid:
                        nc.tensor.matmul(ps_o[:], idf[:], xt[mt][:, t*NTW:(t+1)*NTW],
                                         start=False, stop=True)
                        nc.scalar.copy(ot[:], ps_o[:])
                    else:
                        nc.vector.scalar_tensor_tensor(
                            ot[:], ps_o[:], projb[:, mt:mt+1],
                            xt[mt][:, t*NTW:(t+1)*NTW], AL.add, AL.add)
                    nc.sync.dma_start(o_out[m][mt*128:(mt+1)*128, t*NTW:(t+1)*NTW], ot[:])

            ph_vt_kwf(0)
            ph_qpath(0, 8)
            ph_vt_kwf(1)
            ph_logits(0)
            ph_vlo(0)
            logits_alloc(1)

            def fill_log1(kp):
                if kp % 2 == 0 and kp // 2 < NT:
                    logits_strip(1, kp // 2)
            ph_transp_av(0, filler=fill_log1)
            logits_fin(1)
            ph_vlo(1)
            conv_alloc(0)

            def fill_conv0(kp):
                pt, t = divmod(kp, NT)
                conv_strip(0, pt, t)
            ph_transp_av(1, filler=fill_conv0)
            for pt in range(2):
                conv_corr(0, pt)
            conv_alloc(1)
            # interleave branch-1 conv with branch-0 proj; then pipeline
            # branch-1 proj into branch-1/pt-1 conv via strip-local
            # corrections
            for t in range(NT):
                conv_strip(1, 0, t)
                ph_tail_proj_tile(0, t)
            conv_corr(1, 0)
            for t in range(NT):
                conv_strip(1, 1, t)
                conv_corr_strip(1, 1, t)
                if t >= 1:
                    ph_tail_proj_tile(1, t - 1)
            for t in range(NT - 1, NT):
                ph_tail_proj_tile(1, t)

    nc.compile()
    return nc


# ----------------------------------------------------------------------------
# public entry point
# ----------------------------------------------------------------------------

_CACHE = {}


def kernel(**inputs):
    inputs = {k: np.asarray(v) for k, v in inputs.items()}
    params, flags = _host_precompute(
        **{k: inputs[k] for k in
           ("kv_w", "kv_b", "q_w", "q_b", "proj_w", "proj_b", "dwc_w", "dwc_b",
            "an_bias", "na_bias", "ah_bias", "aw_bias", "ha_bias", "wa_bias")})

    key = tuple(sorted(flags.items()))
    if key not in _CACHE:
        _CACHE[key] = _build(flags)
    nc = _CACHE[key]

    in_maps = _make_in_maps(inputs, params)

    res = run_bass_kernel_spmd(nc, in_maps, core_ids=list(range(B)))
    o1 = np.stack([res.results[b]["o1"].reshape(DIM, H, W) for b in range(B)])
    o2 = np.stack([res.results[b]["o2"].reshape(DIM, H, W) for b in range(B)])
    return o1.astype(np.float32), o2.astype(np.float32)


def _make_in_maps(inputs, params):
    input1, input2, guidmap = inputs["input1"], inputs["input2"], inputs["guidmap"]
    qmeta = params["qmeta"]
    shared = {
        "LOGC_A": params["LOGC_A"], "LOGC_B": params["LOGC_B"],
        "RHSC_A": params["RHSC_A"], "RHSC_B": params["RHSC_B"],
        "ABt": params["ABt"], "WVMM": params["WVMM"], "PW": params["PW"],
        "DIAGW": np.ascontiguousarray(params["DIAGW"].reshape(128, -1)),
        "HM": params["HM"], "IDENT8": params["IDENT8"], "IDF32": params["IDF32"],
        "SMALL_BF": params["SMALL_BF"], "SMALL_F32": params["SMALL_F32"],
    }
    in_maps = []
    for b in range(B):
        g = guidmap[b].reshape(N).astype(np.float32)
        gimg = g.reshape(H, W)
        gblk = gimg.reshape(PS, AGENT, PS, AGENT).transpose(0, 2, 1, 3).reshape(AGENT, 256)
        gcols = g.reshape(NCH, 128).T.copy()
        # agent means in (h,a)-expanded order: gbar128[a + 16*h-ish] follows
        # BLK expansion: gbar128[ha] = gbar[ha % 16]
        gbar = gblk.mean(axis=1)                       # (16,)
        gbar128 = np.tile(gbar, HEADS).reshape(HEADS, AGENT)
        gbar128 = gbar.reshape(1, 16)
        gb = np.zeros(128, np.float32)
        for a in range(16):
            gb[a::16] = gbar[a]
        EG8 = (qmeta["EgC"] * gb[None, :]).astype(F8)   # (8,128)
        qr = qmeta["qrows"]
        u_row = gb * qr[0]
        r_row = None
        if np.any(qr[1]) or np.any(qr[2]):
            u_row = u_row + qr[1]
            r_row = gb * qr[1] + qr[2]
        U128 = np.broadcast_to(u_row.astype(BF), (128, 128))
        R128 = (np.broadcast_to(r_row.astype(BF), (128, 128)) if r_row is not None
                else np.zeros((128, 128), BF))
        in_maps.append({
            "x1": np.ascontiguousarray(input1[b].reshape(DIM, N).astype(BF)),
            "x2": np.ascontiguousarray(input2[b].reshape(DIM, N).astype(BF)),
            "gcols": np.ascontiguousarray(gcols.astype(np.float32)),
            "EG8": np.ascontiguousarray(EG8),
            "U128": np.ascontiguousarray(U128),
            "R128": np.ascontiguousarray(R128),
            **shared,
        })
    return in_maps



# revision 52
# speedup vs baseline: 1.0836x; 1.0559x over previous
"""Trainium2 Bass kernel for nn_Cross_AgentAttention.

Data-parallel over batch B=8 across 8 NeuronCores; params replicated.

Per-core algorithm (feature-major (c, n) layout, exploiting that
q = guidmap @ q_w + q_b is rank-1):
  - v = x @ Wv via fp32r matmuls (TF32-grade, full PE rate), evacuated as
    fp8 hi+lo planes (hi = fp8(v), lo = fp8(v-hi)) with zero-padded y
    borders.
  - agent->kv attention collapses to kw[h,i] = w_h . k_h[i], computed as
    x @ Mkw (folded on host); logits = scale*gbar_a*kw[h,i] + PB, where
    row-constant terms cancel in softmax.  attn = exp(logits) stored fp8.
  - query->agent attention collapses to a rank-1 logit map
    lq[i,(h,a)] = g_i * u[(h,a)] (+r) + ABt; qnt stored fp8.
  - agent_v via fp8 DoubleRow matmuls over transposed attn/v chunks
    (both q-chunks of a pair packed in the two K-tiles of one DR op).
  - depthwise 3x3 conv: every tap is fp8 DoubleRow matmuls against the
    padded v hi/lo planes; weight fp8 residuals get their own paired DR
    ops, so the conv is bf16-grade accurate at fp8 double-pump rate.
  - x-wraparound border corrections batched once per branch.
  - attn-output matmul = one DR op per strip (avsel hi/lo planes).
"""
import numpy as np
import ml_dtypes

import concourse.bass as bass
import concourse.bacc as bacc
import concourse.mybir as mybir
from concourse.tile import TileContext
from concourse.ap import AP
from concourse.bass_utils import run_bass_kernel_spmd

F32 = mybir.dt.float32
F32R = mybir.dt.float32r
BF16 = mybir.dt.bfloat16
FP8 = mybir.dt.float8e4
BF = ml_dtypes.bfloat16
F8 = ml_dtypes.float8_e4m3
DRM = mybir.MatmulPerfMode.DoubleRow

DIM = 256
HEADS = 8
AGENT = 16
H = W = 64
B = 8
N = H * W                 # 4096
HD = DIM // HEADS         # 32
SCALE = HD ** -0.5
PS = 4
NT = 8                    # n-tiles of 512
NTW = N // NT             # 512
NCH = 32                  # n-chunks of 128
ROWS_PER_NT = NTW // W    # 8 image rows per n-tile
VPAD = 128                # zero pad on each side of each v plane
VPLANE = VPAD + N + VPAD  # 4352
# conv tap shifts, row-major (dy,dx) in (-1,0,1)^2
TAPS = [dy * W + dx for dy in (-1, 0, 1) for dx in (-1, 0, 1)]

AL = mybir.AluOpType
AF = mybir.ActivationFunctionType


# ----------------------------------------------------------------------------
# host precompute
# ----------------------------------------------------------------------------

def _bilinear_matrix(n_in, n_out):
    U = np.zeros((n_out, n_in), dtype=np.float64)
    s = n_in / n_out
    for o in range(n_out):
        x = (o + 0.5) * s - 0.5
        x0 = int(np.floor(x))
        t = x - x0
        for i, wt in ((x0, 1.0 - t), (x0 + 1, t)):
            ic = min(max(i, 0), n_in - 1)
            U[o, ic] += wt
    return U.astype(np.float32)


def _host_precompute(kv_w, kv_b, q_w, q_b, proj_w, proj_b, dwc_w, dwc_b,
                     an_bias, na_bias, ah_bias, aw_bias, ha_bias, wa_bias):
    c = DIM
    w = q_w[0]
    beta = q_b
    U = _bilinear_matrix(PS, H)

    # logits-matmul constant operands: logits = LOG^T @ RHS with
    # LOG rows = [Eg-gbar(8, device) | EgC(8) | an_tbl(16) | ahT(64) | awT(64)]
    # RHS rows = [kw(8, device) | kbeta(8, device) | UU(16) | Yind(64) | Xind(64)]
    an_tbl = an_bias.reshape(HEADS * AGENT, PS * PS).T.astype(np.float32)      # (16, 128)
    UU = np.einsum("yr,xc->rcyx", U, U).reshape(PS * PS, N).astype(np.float32)  # (16, 4096)
    ahT = ah_bias[0][..., 0].reshape(HEADS * AGENT, H).T.astype(np.float32)    # (64, 128)
    awT = aw_bias[0][:, :, 0, :].reshape(HEADS * AGENT, W).T.astype(np.float32)
    Yind = np.kron(np.eye(H, dtype=np.float32), np.ones((1, W), np.float32))    # (64, 4096)
    Xind = np.concatenate([np.eye(W, dtype=np.float32)] * H, axis=1)            # (64, 4096)

    na_up = np.einsum("yr,harc,xc->hayx", U, na_bias.reshape(HEADS, AGENT, PS, PS), U)
    ab = na_up.reshape(HEADS, AGENT, N).transpose(0, 2, 1)
    ab = ab + (ha_bias[0] + wa_bias[0]).reshape(HEADS, N, AGENT)
    ABt = ab.transpose(1, 0, 2).reshape(N, HEADS * AGENT).astype(np.float32)

    wk = kv_w[:, :c]
    Mkw = np.stack([(wk[:, h*HD:(h+1)*HD] * w[None, h*HD:(h+1)*HD]).sum(1)
                    for h in range(HEADS)], axis=1)
    Mkb = np.stack([(wk[:, h*HD:(h+1)*HD] * beta[None, h*HD:(h+1)*HD]).sum(1)
                    for h in range(HEADS)], axis=1)
    MM = np.concatenate([Mkw, Mkb], axis=1).astype(np.float32)      # (256, 16)

    hw2 = np.array([(w[h*HD:(h+1)*HD]**2).sum() for h in range(HEADS)], np.float32)
    wb = np.array([(w[h*HD:(h+1)*HD]*beta[h*HD:(h+1)*HD]).sum() for h in range(HEADS)], np.float32)
    bb = np.array([(beta[h*HD:(h+1)*HD]**2).sum() for h in range(HEADS)], np.float32)
    # qrows: [s*hw2 | s*wb | s*bb] repeated per agent -> (1, 384)
    qrows = np.concatenate([np.repeat(SCALE * hw2, AGENT),
                            np.repeat(SCALE * wb, AGENT),
                            np.repeat(SCALE * bb, AGENT)])[None, :].astype(np.float32)

    Wv = kv_w[:, c:].astype(np.float32)                              # (256, 256)
    bv = kv_b[c:].astype(np.float32)

    # fp8 hi/lo planes of Wv and MM, kh halves packed side by side on the
    # same partitions so DoubleRow's j-dim performs the K=256 contraction
    def _kh_pack(W):  # (256, K) -> (128, 2K)
        return np.concatenate([W[0:128], W[128:256]], axis=1)

    Wv_hi = Wv.astype(F8).astype(np.float32)
    Wv_lo = Wv - Wv_hi
    MM_hi = MM.astype(F8).astype(np.float32)
    MM_lo = MM - MM_hi
    WV8 = np.concatenate([_kh_pack(Wv_hi), _kh_pack(Wv_lo),
                          _kh_pack(MM_hi), _kh_pack(MM_lo)], axis=1)  # (128, 1088)

    headmask = np.zeros((HEADS * AGENT, c), np.float32)
    for h in range(HEADS):
        headmask[h*AGENT:(h+1)*AGENT, h*HD:(h+1)*HD] = 1.0

    # EgC: constant rows 8..15 of Eg (selector for the k-beta stream)
    EgC = np.zeros((8, 128), np.float32)
    for h in range(HEADS):
        EgC[h, h*AGENT:(h+1)*AGENT] = SCALE
    HB8 = EgC.copy()   # same pattern masks the gbar broadcast into Eg rows 0..7

    # DIAGW: fp8 hi/lo diagonal tap matrices.
    # layout [k=128, plane(2: hi,lo), tap(10: 0..8 + zero), pt(2), m=128]
    dwc9 = dwc_w.reshape(c, 9).astype(np.float32)
    w_hi = dwc9.astype(F8).astype(np.float32)
    w_lo = (dwc9 - w_hi).astype(F8).astype(np.float32)
    DIAGW = np.zeros((128, 2, 10, 2, 128), np.float32)
    for plane, wsrc in ((0, w_hi), (1, w_lo)):
        for t in range(9):
            for pt in range(2):
                np.fill_diagonal(DIAGW[:, plane, t, pt, :], wsrc[pt*128:(pt+1)*128, t])

    BLK = np.zeros((16, 128), np.float32)                            # gbar -> (h,a) expand
    for a in range(16):
        BLK[a, a::16] = 1.0

    NEG9 = np.zeros((128, 18), np.float32)
    for pt in range(2):
        NEG9[:, pt*9:(pt+1)*9] = -dwc9[pt*128:(pt+1)*128, :]

    projb = np.stack([proj_b[:128], proj_b[128:]], axis=1).astype(np.float32)  # (128, 2)
    bvcol = np.stack([bv[:128], bv[128:]], axis=1).astype(np.float32)          # (128, 2)
    dwbcol = np.stack([dwc_b[:128], dwc_b[128:]], axis=1).astype(np.float32)   # (128, 2)

    flags = dict(
        has_qb=bool(np.any(q_b != 0)),
        has_kvb_v=bool(np.any(bv != 0)),
        has_dwcb=bool(np.any(dwc_b != 0)),
        has_projb=bool(np.any(proj_b != 0)),
    )
    qmeta = dict(qrows=np.concatenate([np.repeat(SCALE * hw2, AGENT),
                                       np.repeat(SCALE * wb, AGENT),
                                       np.repeat(SCALE * bb, AGENT)]).reshape(3, 128),
                 EgC=EgC)

    LOGC_A = np.zeros((128, 128), np.float32)
    LOGC_A[8:16] = EgC
    LOGC_A[16:32] = an_tbl
    LOGC_A[32:96] = ahT
    LOGC_A[96:128] = awT[0:32]
    LOGC_B = np.zeros((128, 128), np.float32)        # plane-1 lhsT, zero-padded
    LOGC_B[0:32] = awT[32:64]
    RHSC_A = np.concatenate([UU, Yind, Xind[0:32]], axis=0)   # (112, 4096)
    RHSC_B = np.zeros((128, N), np.float32)                   # zero-padded plane B
    RHSC_B[0:32] = Xind[32:64]

    # SMALL_BF (128, 144): [ident 0:128 | i16 128:144]
    SMALL_BF = np.zeros((128, 144), np.float32)
    SMALL_BF[:, 0:128] = np.eye(128, dtype=np.float32)
    SMALL_BF[0:16, 128:144] = np.eye(16, dtype=np.float32)
    # SMALL_F32 (128, 24): [neg9 0:18 | projb 18:20 | bvcol 20:22 | dwb 22:24]
    SMALL_F32 = np.concatenate([NEG9, projb, bvcol, dwbcol], axis=1)

    IDENT8 = np.eye(128, dtype=np.float32)
    IDF32 = np.eye(128, dtype=np.float32)

    # ABt pre-arranged to the on-chip layout (128, NCH*128) for a contiguous DMA
    ABt_r = np.ascontiguousarray(
        ABt.reshape(NCH, 128, 128).transpose(1, 0, 2).reshape(128, NCH * 128))

    params = dict(
        LOGC_A=LOGC_A.astype(F8), LOGC_B=LOGC_B.astype(F8),
        RHSC_A=RHSC_A.astype(F8), RHSC_B=RHSC_B.astype(F8),
        ABt=ABt_r.astype(F8),
        WV8=WV8.astype(F8),
        PW=_kh_pack(proj_w.astype(np.float32)).astype(BF),
        DIAGW=DIAGW.astype(F8), HM=headmask.astype(BF),
        IDENT8=IDENT8.astype(F8), IDF32=IDF32.astype(BF),
        SMALL_BF=SMALL_BF.astype(BF), SMALL_F32=SMALL_F32.astype(np.float32),
        ONES2=np.ones((128, 2), F8),
    )
    params["qmeta"] = qmeta
    return params, flags


# ----------------------------------------------------------------------------
# device kernel builder
# ----------------------------------------------------------------------------

def _build(flags):
    nc = bacc.Bacc(None, target_bir_lowering=False, debug=False)

    # ---- DRAM I/O ----
    x_in = [nc.dram_tensor(f"x{m+1}", [128, 4 * N], FP8, kind="ExternalInput") for m in range(2)]
    dGROW = nc.dram_tensor("GROW2", [2, N], BF16, kind="ExternalInput")
    dUR = nc.dram_tensor("UR2", [2, 128], BF16, kind="ExternalInput")
    dLOGAB = nc.dram_tensor("LOGAB", [128, 256], FP8, kind="ExternalInput")
    dRHSA = nc.dram_tensor("RHSC_A", [112, N], FP8, kind="ExternalInput")
    dRHSB = nc.dram_tensor("RHSC_B", [128, N], FP8, kind="ExternalInput")
    dABt = nc.dram_tensor("ABt", [128, NCH * 128], FP8, kind="ExternalInput")
    dWV8 = nc.dram_tensor("WV8", [128, 1088], FP8, kind="ExternalInput")
    dPW = nc.dram_tensor("PW", [128, 2 * DIM], BF16, kind="ExternalInput")
    dDIAGW = nc.dram_tensor("DIAGW", [128, 2 * 10 * 2 * 128], FP8, kind="ExternalInput")
    dHM = nc.dram_tensor("HM", [128, DIM], BF16, kind="ExternalInput")
    dI8 = nc.dram_tensor("IDENT8", [128, 128], FP8, kind="ExternalInput")
    dSBF = nc.dram_tensor("SMALL_BF", [128, 144], BF16, kind="ExternalInput")
    dSF32 = nc.dram_tensor("SMALL_F32", [128, 24], F32, kind="ExternalInput")
    dONES = nc.dram_tensor("ONES2", [128, 2], FP8, kind="ExternalInput")
    o_out = [nc.dram_tensor(f"o{m+1}", [DIM, N], BF16, kind="ExternalOutput") for m in range(2)]

    with TileContext(nc) as tc:
        with (
            tc.tile_pool(name="wpool", bufs=1) as wp,          # weights/consts
            tc.tile_pool(name="big", bufs=1) as bigp,          # big per-branch tensors
            tc.tile_pool(name="xpool", bufs=2) as xp,          # input prefetch
            tc.tile_pool(name="small", bufs=3) as sp,          # rotating small tiles
            tc.tile_pool(name="ps_big", bufs=3, space="PSUM") as psb,    # (128,512)
            tc.tile_pool(name="ps_half", bufs=3, space="PSUM") as psh,   # (128,256)
            tc.tile_pool(name="ps_sm", bufs=2, space="PSUM") as pssm,    # (128,128)
            tc.tile_pool(name="ps_av", bufs=1, space="PSUM") as psav,
        ):
            # ---------------- critical-path DMAs first ----------------
            # tiny q-path seeds first (they head the DVE queue), then the
            # v-matmul operands strip-chunked so the first matmul starts early
            wv8 = wp.tile([128, 1088], FP8)
            nc.sync.dma_start(wv8[:, 0:512], dWV8[:, 0:512])

            xts = [xp.tile([128, 4 * N], FP8, tag=f"x{m}", name=f"xt{m}", bufs=1)
                   for m in range(2)]

            def x_chunk_dma(m, c0, c1):
                # one DMA brings the col slices of all 4 x sections
                # ([xhi_kh0|xhi_kh1|xlo_kh0|xlo_kh1]) together
                nc.sync.dma_start(
                    AP(xts[m][:].tensor, c0, [[4 * N, 128], [N, 4], [1, c1 - c0]]),
                    AP(x_in[m][:].tensor, c0, [[4 * N, 128], [N, 4], [1, c1 - c0]]))

            smallbf = wp.tile([128, 144], BF16)
            grow = wp.tile([2, N], BF16)
            ur = wp.tile([2, 128], BF16)
            smallf = wp.tile([128, 24], F32)
            ones2 = wp.tile([128, 2], FP8)
            # first strip split small so the first matmul starts early
            x_chunk_dma(0, 0, NTW)
            nc.sync.dma_start(wv8[:, 512:1088], dWV8[:, 512:1088])
            x_chunk_dma(0, NTW, 2 * NTW)
            for t2 in range(1, NT // 2):
                x_chunk_dma(0, t2 * 2 * NTW, (t2 + 1) * 2 * NTW)
            nc.sync.dma_start(smallbf[:], dSBF[:])
            nc.sync.dma_start(grow[:], dGROW[:])
            nc.sync.dma_start(ur[:], dUR[:])
            nc.sync.dma_start(smallf[:], dSF32[:])
            nc.sync.dma_start(ones2[:], dONES[:])

            logab = wp.tile([128, 256], FP8)
            nc.sync.dma_start(logab[:], dLOGAB[:])

            ident8 = wp.tile([128, 128], FP8)
            nc.sync.dma_start(ident8[:], dI8[:])
            abt = bigp.tile([128, NCH * 128], FP8, tag="attn", bufs=2)
            nc.sync.dma_start(abt[:], dABt[:])

            # rhs2 (shared by both branches): [kw | consts] plane A, plane B
            # consts; branch-1 kw rows overwrite branch-0's after its logits
            rhs2 = bigp.tile([128, 2 * N], FP8, tag="rhsa", bufs=1)
            nc.sync.dma_start(rhs2[16:128, 0:N], dRHSA[:])
            nc.sync.dma_start(rhs2[:, N:2 * N], dRHSB[:])

            hm = wp.tile([128, DIM], BF16)
            nc.sync.dma_start(hm[:], dHM[:])

            for t2 in range(NT // 2):
                x_chunk_dma(1, t2 * 2 * NTW, (t2 + 1) * 2 * NTW)
            ident = smallbf[:, 0:128]
            neg9 = smallf[:, 0:18]
            projb = smallf[:, 18:20]
            bvcol = smallf[:, 20:22]
            dwbcol = smallf[:, 22:24]

            diagw = wp.tile([128, 2 * 10 * 2 * 128], FP8)
            nc.sync.dma_start(diagw[:], dDIAGW[:])
            pw = wp.tile([128, 2 * DIM], BF16)   # PW as 2 K-half tiles side by side
            nc.sync.dma_start(pw[:], dPW[:])

            def dw_ap(plane, tap, pt, jstride, jn=2):
                # lhsT view [128, 2, 128] into diagw
                off = plane * 2560 + tap * 256 + pt * 128
                return AP(diagw[:].tensor, off, [[5120, 128], [jstride, jn], [1, 128]])

            # ---------------- per-branch pipeline, phase-interleaved ----------------
            st = [dict(), dict()]
            qh = {}
            ps_av2 = psav.tile([128, 512], F32, tag="av")

            qnt_t = wp.tile([128, N], FP8, name="qnt")   # (h,a) x n, normalized q-attn

            def ph_qpath(g0, g1):
                qnt = qnt_t
                for grp in range(g0, g1):
                    # rank-1 logits on PE: lq[i,ha] = g_i*u[ha] + r[ha] + ABt[i,ha]
                    ps_lq = psh.tile([128, 512], F32, tag="half", name="ps_lq")
                    for jj in range(4):
                        j = grp * 4 + jj
                        sl = ps_lq[:, jj*128:(jj+1)*128]
                        nc.tensor.matmul(sl, grow[:, j*128:(j+1)*128], ur[:],
                                         start=True, stop=False)
                        nc.tensor.matmul(sl, ident8[:], abt[:, j*128:(j+1)*128],
                                         start=False, stop=True)
                    lqg = sp.tile([128, 512], BF16, tag="lq", bufs=2, name="lqg")
                    nc.scalar.activation(lqg[:], ps_lq[:], AF.Exp)
                    sqg = sp.tile([128, 32], F32, tag="sq", bufs=2, name="sqg")
                    nc.vector.tensor_reduce(sqg[:], lqg[:].rearrange("p (g b) -> p g b", b=16),
                                            mybir.AxisListType.X, AL.add)
                    rqg = sp.tile([128, 32], F32, tag="rq", bufs=2, name="rqg")
                    nc.vector.reciprocal(rqg[:], sqg[:])
                    qng = lqg
                    nc.vector.tensor_tensor(
                        qng[:].rearrange("p (g b) -> p g b", b=16),
                        qng[:].rearrange("p (g b) -> p g b", b=16),
                        rqg[:].unsqueeze(2).broadcast_to([128, 32, 16]), AL.mult)
                    qngs[grp] = qng
                    # transposes pipelined one grp behind the DVE chain
                    if grp > g0:
                        qp_transp(grp - 1)
                if g1 > g0:
                    qp_transp(g1 - 1)
                qh["qnt"] = qnt

            qngs = {}

            def qp_transp(grp):
                qng = qngs.pop(grp)
                for half in range(2):
                    ps_q = pssm.tile([128, 256], BF16, tag="sm", name="ps_q")
                    for q2 in range(2):
                        jj = half * 2 + q2
                        nc.tensor.transpose(ps_q[:, q2*128:(q2+1)*128],
                                            qng[:, jj*128:(jj+1)*128], ident)
                    base = (grp * 4 + half * 2) * 128
                    if half == 0:
                        nc.vector.tensor_copy(qnt_t[:, base:base+256], ps_q[:])
                    else:
                        nc.scalar.copy(qnt_t[:, base:base+256], ps_q[:])

            def vt_alloc(m):
                # v8: fp8 hi/lo planes with zero-padded y borders.  The
                # psum is evacuated once to bf16 (ACT/DVE); the fp8 hi copy
                # and the deferred lo = v - hi both run on Pool (SBUF-only).
                v8 = [bigp.tile([128, 2 * VPLANE], FP8, tag=f"v8{pt}", name=f"v8{m}{pt}", bufs=2)
                      for pt in range(2)]
                vbf = [bigp.tile([128, N], BF16, tag=f"pre{pt}", name=f"vbf{m}{pt}", bufs=2)
                       for pt in range(2)]
                for pt in range(2):
                    for plane in range(2):
                        nc.gpsimd.memset(v8[pt][:, plane*VPLANE:plane*VPLANE + VPAD], 0.0)
                        nc.gpsimd.memset(v8[pt][:, plane*VPLANE + VPAD + N:(plane+1)*VPLANE], 0.0)
                st[m]["v8"] = v8
                st[m]["vbf"] = vbf

            def xt_ap(m, plane, t, w=NTW):
                # rhs AP over x: j packs the two kh K-tiles of one hi/lo plane
                return AP(xts[m][:].tensor, plane * 2 * N + t * NTW,
                          [[4 * N, 128], [N, 2], [1, w]])

            def wv_ap(plane, pt):
                # lhsT AP over Wv: j packs the two kh K-tiles
                return AP(wv8[:].tensor, plane * 512 + pt * 128,
                          [[1088, 128], [256, 2], [1, 128]])

            def mm_ap(plane):
                return AP(wv8[:].tensor, 1024 + plane * 32,
                          [[1088, 128], [16, 2], [1, 16]])

            def vt_unit(m, t):
                v8, vbf = st[m]["v8"], st[m]["vbf"]
                for pt in range(2):
                    ps_v = psb.tile([128, NTW], F32, tag="big")
                    nc.tensor.matmul(ps_v[:], wv_ap(0, pt), xt_ap(m, 0, t),
                                     start=True, stop=False, perf_mode=DRM)
                    nc.tensor.matmul(ps_v[:], wv_ap(0, pt), xt_ap(m, 1, t),
                                     start=False, stop=False, perf_mode=DRM,
                                     skip_group_check=True)
                    nc.tensor.matmul(ps_v[:], wv_ap(1, pt), xt_ap(m, 0, t),
                                     start=False, stop=True, perf_mode=DRM,
                                     skip_group_check=True)
                    hi = v8[pt][:, VPAD + t*NTW: VPAD + (t+1)*NTW]
                    vb = vbf[pt][:, t*NTW:(t+1)*NTW]
                    if flags["has_kvb_v"]:
                        nc.vector.tensor_scalar(vb, ps_v[:], bvcol[:, pt:pt+1], None, AL.add)
                    elif (m == 0 and t % 2 == 0) or (m == 1 and pt == 1):
                        nc.scalar.copy(vb, ps_v[:])
                    else:
                        nc.vector.tensor_copy(vb, ps_v[:])
                    nc.gpsimd.tensor_copy(hi, vb)
                ps_k = psh.tile([16, NTW], F32, tag="half")
                nc.tensor.matmul(ps_k[:], mm_ap(0), xt_ap(m, 0, t),
                                 start=True, stop=False, perf_mode=DRM)
                nc.tensor.matmul(ps_k[:], mm_ap(0), xt_ap(m, 1, t),
                                 start=False, stop=False, perf_mode=DRM,
                                 skip_group_check=True)
                nc.tensor.matmul(ps_k[:], mm_ap(1), xt_ap(m, 0, t),
                                 start=False, stop=True, perf_mode=DRM,
                                 skip_group_check=True)
                if m == 0 and t % 2 == 0:
                    nc.vector.tensor_copy(rhs2[0:16, t*NTW:(t+1)*NTW], ps_k[:])
                else:
                    nc.scalar.copy(rhs2[0:16, t*NTW:(t+1)*NTW], ps_k[:])

            def ph_vlo(m):
                # deferred lo-plane: Pool computes v - hi in the background
                v8, vbf = st[m]["v8"], st[m]["vbf"]
                for pt in range(2):
                    for t in range(NT):
                        lo = v8[pt][:, VPLANE + VPAD + t*NTW: VPLANE + VPAD + (t+1)*NTW]
                        hi = v8[pt][:, VPAD + t*NTW: VPAD + (t+1)*NTW]
                        nc.gpsimd.tensor_tensor(lo, vbf[pt][:, t*NTW:(t+1)*NTW], hi,
                                                AL.subtract)

            def logits_alloc(m):
                # attnT: exp(logits) in TRANSPOSED (px, ha) chunk layout,
                # ready to be the agent_v lhsT without any attn transposes
                st[m]["attn"] = bigp.tile([128, N], FP8, tag="attn", name=f"attn{m}", bufs=2)

            def logitsT_unit(m, u):
                # 4 px-chunks: logitsT[px, ha] via DR (j packs the
                # [rhs2 plane A | plane B] x [loga | logb] contraction)
                attnT = st[m]["attn"]
                ps_l = psb.tile([128, 512], F32, tag="big", name="ps_lt")
                for q in range(4):
                    c = u * 4 + q
                    lhs = AP(rhs2[:].tensor, c * 128, [[2 * N, 128], [N, 2], [1, 128]])
                    rhs = AP(logab[:].tensor, 0, [[256, 128], [128, 2], [1, 128]])
                    nc.tensor.matmul(ps_l[:, q*128:(q+1)*128], lhs, rhs,
                                     start=True, stop=True, perf_mode=DRM)
                nc.scalar.activation(attnT[:, u*512:(u+1)*512], ps_l[:], AF.Exp)

            def ph_logits(m):
                logits_alloc(m)
                for u in range(NT):
                    logitsT_unit(m, u)

            def ph_transp_av(m, filler=None):
                attnT, v8 = st[m]["attn"], st[m]["v8"]
                ps_av = ps_av2[:, m*DIM:(m+1)*DIM]
                ps_s1 = pssm.tile([128, 8], F32, tag="sm", name=f"s1ps{m}")
                for kp in range(NCH // 2):
                    if filler is not None:
                        filler(kp)
                    # fp8 transpose must write element-step-2 psum; layout per
                    # q (stride-2 slots): [vT0(256) | vT1(256)]
                    ps_t = psh.tile([128, 1024], FP8, tag="half")
                    for q in range(2):
                        k = kp * 2 + q
                        for pt in range(2):
                            nc.tensor.transpose(
                                AP(ps_t[:].tensor, q*512 + pt*256,
                                   [[1024, 128], [2, 128]]),
                                v8[pt][:, VPAD + k*128:VPAD + (k+1)*128], ident8[:])
                    # evac keeps the stride-2 psum image; the DVE path copies
                    # all bytes as packed uint16 (2x mode), the matmul reads
                    # the SBUF copy as fp8 at stride 2
                    tv = sp.tile([128, 1024], FP8, tag="tv", name=f"tv{m}", bufs=2)
                    src16 = AP(ps_t[:].tensor, 0,
                               [[1024, 128], [1, 1024]]).bitcast(mybir.dt.uint16)
                    dst16 = AP(tv[:].tensor, 0,
                               [[1024, 128], [1, 1024]]).bitcast(mybir.dt.uint16)
                    nc.vector.tensor_copy(dst16, src16)
                    # agent_v DR op: lhsT = attnT chunks (j packs the two
                    # q-chunks), rhs = vT planes; plus a tiny ones-matmul
                    # accumulating the softmax denominators
                    lhsa = AP(attnT[:].tensor, kp * 256, [[N, 128], [128, 2], [1, 128]])
                    rhs = AP(tv[:].tensor, 0, [[1024, 128], [512, 2], [2, 256]])
                    nc.tensor.matmul(ps_av[:], lhsa, rhs,
                                     start=(kp == 0), stop=(kp == NCH // 2 - 1),
                                     perf_mode=DRM)
                    nc.tensor.matmul(
                        ps_s1[:, 0:1], lhsa,
                        AP(ones2[:].tensor, 0, [[2, 128], [1, 2], [1, 1]]),
                        start=(kp == 0), stop=(kp == NCH // 2 - 1),
                        perf_mode=DRM)
                rs1 = bigp.tile([128, 1], F32, tag="rs1", name=f"rs1{m}", bufs=2)
                nc.vector.reciprocal(rs1[:], ps_s1[:, 0:1])
                avf = sp.tile([128, DIM], BF16, tag="avf", name=f"avf{m}", bufs=2)
                nc.vector.scalar_tensor_tensor(avf[:], ps_av[:], rs1[:], hm[:],
                                               AL.mult, AL.mult)
                avsel = bigp.tile([128, 2 * DIM], FP8, tag="avsel", name=f"avsel{m}", bufs=2)
                nc.vector.tensor_copy(avsel[:, 0:DIM], avf[:])
                nc.vector.scalar_tensor_tensor(avsel[:, DIM:2*DIM], avsel[:, 0:DIM], -1.0,
                                               avf[:], AL.mult, AL.add)
                st[m]["avsel"] = avsel

            # tap pairs for the 2-per-op DR packing
            PAIRS4 = [(0, 1), (2, 3), (4, 5), (6, 7)]

            def conv_alloc(m):
                st[m]["pre"] = [bigp.tile([128, N], BF16, tag=f"pre{pt}", name=f"pre{m}{pt}", bufs=2)
                                for pt in range(2)]

            def conv_strip(m, pt, t):
                v8, avsel, pre = st[m]["v8"], st[m]["avsel"], st[m]["pre"]
                qnt = qh["qnt"]
                ps_n = psb.tile([128, NTW], F32, tag="big")
                base = VPAD + t * NTW
                # tap pairs: plane 0 = [w_hi(A)|w_hi(B)] @ [v_hi(sA)|v_hi(sB)],
                # plane 1 = [w_lo(A)|w_lo(B)] @ same (w_lo*v_hi correction)
                first = True
                for plane in range(2):
                    for ka, kb in PAIRS4:
                        rhs = AP(v8[pt][:].tensor, base + TAPS[ka],
                                 [[2 * VPLANE, 128], [TAPS[kb] - TAPS[ka], 2],
                                  [1, NTW]])
                        nc.tensor.matmul(ps_n[:], dw_ap(plane, ka, pt, 256), rhs,
                                         start=first, stop=False,
                                         perf_mode=DRM, skip_group_check=True)
                        first = False
                # tap 8: [w_hi(8) | w_lo(8)] @ [v_hi(s8) | v_hi(s8)]
                rhs = AP(v8[pt][:].tensor, base + TAPS[8],
                         [[2 * VPLANE, 128], [0, 2], [1, NTW]])
                nc.tensor.matmul(ps_n[:], dw_ap(0, 8, pt, 2560), rhs,
                                 start=False, stop=False,
                                 perf_mode=DRM, skip_group_check=True)
                # tap pairs hi*lo: [w_hi(A)|w_hi(B)] @ [v_lo(sA)|v_lo(sB)]
                # (tap 8's hi*lo term is dropped: ~1% of one tap's magnitude)
                for ka, kb in PAIRS4:
                    rhs = AP(v8[pt][:].tensor, VPLANE + base + TAPS[ka],
                             [[2 * VPLANE, 128], [TAPS[kb] - TAPS[ka], 2],
                              [1, NTW]])
                    nc.tensor.matmul(ps_n[:], dw_ap(0, ka, pt, 256), rhs,
                                     start=False, stop=False,
                                     perf_mode=DRM, skip_group_check=True)
                # attn output: [avsel_hi | avsel_lo] @ [qnt | qnt]
                lhs = AP(avsel[:].tensor, pt * 128, [[512, 128], [256, 2], [1, 128]])
                rhs = AP(qnt[:].tensor, t * NTW, [[N, 128], [0, 2], [1, NTW]])
                nc.tensor.matmul(ps_n[:], lhs, rhs,
                                 start=False, stop=True,
                                 perf_mode=DRM, skip_group_check=True)
                sl = pre[pt][:, t*NTW:(t+1)*NTW]
                if flags["has_dwcb"]:
                    nc.vector.tensor_scalar(sl, ps_n[:],
                                            dwbcol[:, pt:pt+1], None, AL.add)
                elif t % 2 == 0:
                    nc.scalar.copy(sl, ps_n[:])
                else:
                    nc.vector.tensor_copy(sl, ps_n[:])

            # x-wraparound border corrections, batched over the full image.
            # With the zero-padded planes the flat-shift taps run unclipped,
            # so every row whose wrapped read lands on real data needs a fix:
            # col 63 of pre[y] wrongly got w_k * v[y+dy+1, 0] (dx=+1 taps),
            # col 0 wrongly got w_k * v[y+dy-1, 63] (dx=-1 taps).
            CORR_HI = ((2, 0, 0, 64), (5, 1, 0, 63), (8, 2, 0, 62))   # k, off, ya, yb
            CORR_LO = ((0, -2, 2, 64), (3, -1, 1, 64), (6, 0, 0, 64))

            def conv_corr_strip(m, pt, t):
                # strip-local x-wrap corrections (rows t*8 .. t*8+8)
                v8, pre = st[m]["v8"], st[m]["pre"]
                pre3 = pre[pt][:].rearrange("p (y x) -> p y x", y=H)
                r0, r1 = t * ROWS_PER_NT, (t + 1) * ROWS_PER_NT
                for xe, corr in ((63, CORR_HI), (0, CORR_LO)):
                    for k, off, ya, yb in corr:
                        ya2, yb2 = max(ya, r0), min(yb, r1)
                        if yb2 <= ya2:
                            continue
                        for plane in range(2):
                            nc.vector.scalar_tensor_tensor(
                                pre3[:, ya2:yb2, xe:xe+1],
                                AP(v8[pt][:].tensor,
                                   plane * VPLANE + VPAD + (ya2 + off) * W + (63 - xe),
                                   [[2 * VPLANE, 128], [W, yb2 - ya2], [1, 1]]),
                                neg9[:, pt*9 + k:pt*9 + k + 1],
                                pre3[:, ya2:yb2, xe:xe+1],
                                AL.mult, AL.add)

            def conv_corr(m, pt):
                v8, pre = st[m]["v8"], st[m]["pre"]
                pre3 = pre[pt][:].rearrange("p (y x) -> p y x", y=H)
                for xe, corr in ((63, CORR_HI), (0, CORR_LO)):
                    for k, off, ya, yb in corr:
                        for plane in range(2):
                            nc.vector.scalar_tensor_tensor(
                                pre3[:, ya:yb, xe:xe+1],
                                AP(v8[pt][:].tensor,
                                   plane * VPLANE + VPAD + (ya + off) * W + (63 - xe),
                                   [[2 * VPLANE, 128], [W, yb - ya], [1, 1]]),
                                neg9[:, pt*9 + k:pt*9 + k + 1],
                                pre3[:, ya:yb, xe:xe+1],
                                AL.mult, AL.add)

            def ph_tail_proj_tile(m, t):
                pre = st[m]["pre"]
                ot2 = sp.tile([128, 2 * NTW], BF16, tag="ot", name=f"ot{m}", bufs=3)
                for mt in range(2):
                    ps_o = psh.tile([128, NTW], F32, tag="half")
                    for kh in range(2):
                        nc.tensor.matmul(
                            ps_o[:], pw[:, kh*DIM + mt*128: kh*DIM + (mt+1)*128],
                            pre[kh][:, t*NTW:(t+1)*NTW],
                            start=(kh == 0), stop=False)
                    # residual: ident8 DR over the [hi|lo] planes of this mt half
                    rhs_r = AP(xts[m][:].tensor, mt * N + t * NTW,
                               [[4 * N, 128], [2 * N, 2], [1, NTW]])
                    lhs_r = AP(ident8[:].tensor, 0, [[128, 128], [0, 2], [1, 128]])
                    nc.tensor.matmul(ps_o[:], lhs_r, rhs_r, start=False, stop=True,
                                     perf_mode=DRM, skip_group_check=True)
                    ot = ot2[:, mt*NTW:(mt+1)*NTW]
                    if flags["has_projb"]:
                        nc.vector.tensor_scalar(ot, ps_o[:], projb[:, mt:mt+1],
                                                None, AL.add)
                    elif mt == 0:
                        nc.vector.tensor_copy(ot, ps_o[:])
                    else:
                        nc.scalar.copy(ot, ps_o[:])
                # one DMA stores both mt halves of the tile
                nc.sync.dma_start(
                    AP(o_out[m][:].tensor, t * NTW, [[N, 128], [128 * N, 2], [1, NTW]]),
                    AP(ot2[:].tensor, 0, [[2 * NTW, 128], [NTW, 2], [1, NTW]]))

            vt_alloc(0)
            for t in range(NT):
                vt_unit(0, t)
            ph_qpath(0, 8)
            ph_logits(0)
            ph_vlo(0)
            vt_alloc(1)
            logits_alloc(1)

            # branch-1 v/kw matmuls + logits interleave into branch-0's
            # transpose/agent_v phase as its PE filler
            def fill_vt1(kp):
                if kp % 2 == 0 and kp // 2 < NT:
                    t = kp // 2
                    vt_unit(1, t)
                    logitsT_unit(1, t)
            ph_transp_av(0, filler=fill_vt1)
            ph_vlo(1)
            conv_alloc(0)

            def fill_conv0(kp):
                pt, t = divmod(kp, NT)
                conv_strip(0, pt, t)
            ph_transp_av(1, filler=fill_conv0)
            for pt in range(2):
                conv_corr(0, pt)
            conv_alloc(1)
            # interleave branch-1 conv with branch-0 proj; then pipeline
            # branch-1 proj into branch-1/pt-1 conv via strip-local
            # corrections
            for t in range(NT):
                conv_strip(1, 0, t)
                ph_tail_proj_tile(0, t)
            conv_corr(1, 0)
            for t in range(NT):
                conv_strip(1, 1, t)
                conv_corr_strip(1, 1, t)
                if t >= 2:
                    ph_tail_proj_tile(1, t - 2)
            for t in range(NT - 2, NT):
                ph_tail_proj_tile(1, t)

    nc.compile()
    return nc


# ----------------------------------------------------------------------------
# public entry point
# ----------------------------------------------------------------------------

_CACHE = {}


def kernel(**inputs):
    inputs = {k: np.asarray(v) for k, v in inputs.items()}
    params, flags = _host_precompute(
        **{k: inputs[k] for k in
           ("kv_w", "kv_b", "q_w", "q_b", "proj_w", "proj_b", "dwc_w", "dwc_b",
            "an_bias", "na_bias", "ah_bias", "aw_bias", "ha_bias", "wa_bias")})

    key = tuple(sorted(flags.items()))
    if key not in _CACHE:
        _CACHE[key] = _build(flags)
    nc = _CACHE[key]

    in_maps = _make_in_maps(inputs, params)

    res = run_bass_kernel_spmd(nc, in_maps, core_ids=list(range(B)))
    o1 = np.stack([res.results[b]["o1"].reshape(DIM, H, W) for b in range(B)])
    o2 = np.stack([res.results[b]["o2"].reshape(DIM, H, W) for b in range(B)])
    return o1.astype(np.float32), o2.astype(np.float32)


def _x_hilo(x):
    # (256, N) f32 -> (128, 4N) fp8 [xhi_kh0 | xhi_kh1 | xlo_kh0 | xlo_kh1]
    xf = x.reshape(DIM, N).astype(np.float32)
    xhi = xf.astype(F8).astype(np.float32)
    xlo = (xf - xhi).astype(F8)
    xhi = xhi.astype(F8)
    return np.ascontiguousarray(np.concatenate(
        [xhi[0:128], xhi[128:256], xlo[0:128], xlo[128:256]], axis=1))


def _make_in_maps(inputs, params):
    input1, input2, guidmap = inputs["input1"], inputs["input2"], inputs["guidmap"]
    qmeta = params["qmeta"]
    shared = {
        "RHSC_A": params["RHSC_A"], "RHSC_B": params["RHSC_B"],
        "ABt": params["ABt"], "WV8": params["WV8"], "PW": params["PW"],
        "DIAGW": np.ascontiguousarray(params["DIAGW"].reshape(128, -1)),
        "HM": params["HM"], "IDENT8": params["IDENT8"],
        "SMALL_BF": params["SMALL_BF"], "SMALL_F32": params["SMALL_F32"],
        "ONES2": params["ONES2"],
    }
    in_maps = []
    for b in range(B):
        g = guidmap[b].reshape(N).astype(np.float32)
        gimg = g.reshape(H, W)
        gblk = gimg.reshape(PS, AGENT, PS, AGENT).transpose(0, 2, 1, 3).reshape(AGENT, 256)
        gcols = g.reshape(NCH, 128).T.copy()
        # agent means in (h,a)-expanded order: gbar128[a + 16*h-ish] follows
        # BLK expansion: gbar128[ha] = gbar[ha % 16]
        gbar = gblk.mean(axis=1)                       # (16,)
        gbar128 = np.tile(gbar, HEADS).reshape(HEADS, AGENT)
        gbar128 = gbar.reshape(1, 16)
        gb = np.zeros(128, np.float32)
        for a in range(16):
            gb[a::16] = gbar[a]
        EG8 = (qmeta["EgC"] * gb[None, :]).astype(F8)   # (8,128)
        qr = qmeta["qrows"]
        u_row = gb * qr[0]
        r_row = np.zeros(128, np.float32)
        if np.any(qr[1]) or np.any(qr[2]):
            u_row = u_row + qr[1]
            r_row = gb * qr[1] + qr[2]
        GROW2 = np.stack([g, np.ones(N, np.float32)]).astype(BF)
        UR2 = np.stack([u_row, r_row]).astype(BF)
        LOGAB = np.concatenate(
            [np.vstack([EG8.astype(F8), params["LOGC_A"][8:128]]),
             params["LOGC_B"]], axis=1)   # (128, 256) fp8
        in_maps.append({
            "x1": _x_hilo(input1[b]),
            "x2": _x_hilo(input2[b]),
            "LOGAB": np.ascontiguousarray(LOGAB),
            "GROW2": np.ascontiguousarray(GROW2),
            "UR2": np.ascontiguousarray(UR2),
            **shared,
        })
    return in_maps

